# revision 9
# baseline (speedup 1.0000x reference)
"""MultiHeadAttentionDecoder kernel for 8 Trainium2 NeuronCores.

Sharding: 8-way tensor parallel over heads (2 heads per core), both batches
and all queries on every core.  Each core:
  - projects Q/K/V for its 2 heads (transposed layouts via PE transpose),
  - runs causal flash-attention (scores kept transposed: [keys, queries]),
    softmax without max-subtraction (scores are O(1) bounded), denominator
    via a ones-column appended to V,
  - computes its partial output projection (row-shard of W_out).
Host sums the 8 partial projections (+bias) and concatenates past K/V with
the new K/V computed on device.  Big matmuls run as float32r (~1e-4 rel).
"""

import os
import sys

import numpy as np

sys.path.insert(0, "/opt/trn_rl_repo")

B, H, TN, TP, HS, D = 2, 16, 2048, 2048, 64, 1024
TT = TN + TP                     # 4096 total keys
NCORES = 8
HPC = H // NCORES                # 2 heads per core
KT_N = TT // 128                 # 32 key tiles of 128
GRP = 3                          # score k-tiles per exp group

_cache = {}


def _build_program():
    import concourse.mybir as mybir
    from concourse import bacc
    from concourse.masks import make_identity
    from concourse.tile import TileContext

    F32 = mybir.dt.float32
    F32R = mybir.dt.float32r
    Exp = mybir.ActivationFunctionType.Exp
    MUL = mybir.AluOpType.mult

    nc = bacc.Bacc("TRN2", target_bir_lowering=False, debug=False,
                   num_devices=NCORES)

    xh = nc.dram_tensor("xh", [B * TN, 128], F32, kind="ExternalInput")
    pk = nc.dram_tensor("pk", [B, HPC, TP, HS], F32, kind="ExternalInput")
    pv = nc.dram_tensor("pv", [B, HPC, TP, HS], F32, kind="ExternalInput")
    wk2 = nc.dram_tensor("wk2", [128, HS], F32, kind="ExternalInput")
    wq2 = nc.dram_tensor("wq2", [128, HS], F32, kind="ExternalInput")
    wv2 = nc.dram_tensor("wv2", [128, HS], F32, kind="ExternalInput")
    wo = nc.dram_tensor("wo", [128, D], F32, kind="ExternalInput")
    causal = nc.dram_tensor("causal", [128, 896], F32, kind="ExternalInput")
    pout = nc.dram_tensor("pout", [B * TN, D], F32, kind="ExternalOutput")
    dbg_kt = nc.dram_tensor("dbg_kt", [128, TT], F32, kind="ExternalOutput")
    dbg_qt = nc.dram_tensor("dbg_qt", [128, TN], F32, kind="ExternalOutput")
    dbg_dp = nc.dram_tensor("dbg_dp", [128, 512], F32, kind="ExternalOutput")
    dbg_oh = nc.dram_tensor("dbg_oh", [128, 512], F32, kind="ExternalOutput")
    dbg_rb = nc.dram_tensor("dbg_rb", [128, 512], F32, kind="ExternalOutput")
    dbg_ohn = nc.dram_tensor("dbg_ohn", [128, 512], F32, kind="ExternalOutput")
    dbg_pt = nc.dram_tensor("dbg_pt", [128, 1536], F32, kind="ExternalOutput")
    knew = nc.dram_tensor("knew", [B, HPC, TN, HS], F32, kind="ExternalOutput")
    vnew = nc.dram_tensor("vnew", [B, HPC, TN, HS], F32, kind="ExternalOutput")

    from contextlib import ExitStack
    with TileContext(nc) as tc, ExitStack() as ctx:
        perm = ctx.enter_context(tc.tile_pool(name="perm", bufs=1))
        work = ctx.enter_context(tc.tile_pool(name="work", bufs=3))
        # --- constants ---
        ident = perm.tile([128, 128], F32, tag="ident")
        make_identity(nc, ident[:])
        wk2r = perm.tile([128, HS], F32R, tag="wk2r")
        wq2r = perm.tile([128, HS], F32R, tag="wq2r")
        wv2r = perm.tile([128, HS], F32R, tag="wv2r")
        wor = perm.tile([128, D], F32R, tag="wor")
        caus = perm.tile([128, 896], F32R, tag="caus")
        nc.gpsimd.dma_start(out=wk2r[:], in_=wk2[:])
        nc.gpsimd.dma_start(out=wq2r[:], in_=wq2[:])
        nc.gpsimd.dma_start(out=wv2r[:], in_=wv2[:])
        nc.gpsimd.dma_start(out=wor[:], in_=wo[:])
        nc.gpsimd.dma_start(out=caus[:], in_=causal[:])
        ones4 = perm.tile([128, 4], F32, tag="ones4")
        nc.gpsimd.memset(ones4[:], 1.0)
        ones8 = perm.tile([128, 8], F32, tag="ones8")
        nc.gpsimd.memset(ones8[:], 1.0)

        # --- persistent activations ---
        XT = [perm.tile([128, TN], F32R, tag=f"XT{b}", name=f"XT{b}") for b in range(B)]
        KT = [perm.tile([128, TT], F32R, tag=f"KT{b}", name=f"KT{b}") for b in range(B)]
        QT = [perm.tile([128, TN], F32R, tag=f"QT{b}", name=f"QT{b}") for b in range(B)]
        VT = [[perm.tile([128, KT_N * (HS + 1)], F32R, tag=f"VT{b}{h}", name=f"VT{b}{h}")
               for h in range(HPC)] for b in range(B)]

        def vt_view(b, h):
            return VT[b][h][:].rearrange("p (kt c) -> p kt c", c=HS + 1)

        # ---------------- Phase A: projections / layouts ----------------
        with tc.tile_pool(name="psA", bufs=2, space="PSUM") as psA:
            for b in range(B):
                # X^T for this core's 128 embed dims (2 heads x 64)
                for t in range(TN // 128):
                    xs = work.tile([128, 128], F32, tag="xs")
                    nc.sync.dma_start(out=xs[:], in_=xh[b * TN + t * 128:
                                                       b * TN + (t + 1) * 128, :])
                    tp = psA.tile([128, 128], F32, tag="tp")
                    nc.tensor.transpose(tp[:], xs[:], ident[:])
                    nc.vector.tensor_copy(XT[b][:, t * 128:(t + 1) * 128], tp[:])
                # past K^T (head pair stacked on partitions)
                for t in range(TP // 128):
                    ks = work.tile([128, 128], F32, tag="ks")
                    nc.sync.dma_start(out=ks[:, 0:64],
                                      in_=pk[b, 0, t * 128:(t + 1) * 128, :])
                    nc.sync.dma_start(out=ks[:, 64:128],
                                      in_=pk[b, 1, t * 128:(t + 1) * 128, :])
                    tp = psA.tile([128, 128], F32, tag="tp")
                    nc.tensor.transpose(tp[:], ks[:], ident[:])
                    nc.vector.tensor_copy(KT[b][:, t * 128:(t + 1) * 128], tp[:])
                # new K^T and Q^T (col-tiled head pair)
                for kc in range(TN // 512):
                    sl = slice(kc * 512, (kc + 1) * 512)
                    nsl = slice(TP + kc * 512, TP + (kc + 1) * 512)
                    kp0 = psA.tile([64, 512], F32, tag="kqp", name="kp0")
                    kp1 = psA.tile([64, 512], F32, tag="kqp", name="kp1")
                    nc.tensor.matmul(kp0[:], wk2r[0:64, :], XT[b][0:64, sl],
                                     start=True, stop=True, tile_position=(0, 0))
                    nc.tensor.matmul(kp1[:], wk2r[64:128, :],
                                     XT[b][64:128, sl], start=True, stop=True,
                                     tile_position=(64, 0))
                    nc.vector.tensor_copy(KT[b][0:64, nsl], kp0[:])
                    nc.vector.tensor_copy(KT[b][64:128, nsl], kp1[:])
                    qp0 = psA.tile([64, 512], F32, tag="kqp", name="qp0")
                    qp1 = psA.tile([64, 512], F32, tag="kqp", name="qp1")
                    nc.tensor.matmul(qp0[:], wq2r[0:64, :], XT[b][0:64, sl],
                                     start=True, stop=True, tile_position=(0, 0))
                    nc.tensor.matmul(qp1[:], wq2r[64:128, :],
                                     XT[b][64:128, sl], start=True, stop=True,
                                     tile_position=(64, 0))
                    nc.vector.tensor_copy(QT[b][0:64, sl], qp0[:])
                    nc.vector.tensor_copy(QT[b][64:128, sl], qp1[:])
                # past V into V~ tiles (fp32r cast during SWDGE DMA) + ones col
                for h in range(HPC):
                    for g in range(4):
                        vv = vt_view(b, h)[:, g * 4:(g + 1) * 4, :]
                        src = pv[b, h, g * 512:(g + 1) * 512, :].rearrange(
                            "(kt p) d -> p kt d", p=128)
                        nc.gpsimd.dma_start(out=vv[:, :, 0:HS], in_=src)
                        nc.vector.tensor_copy(
                            vv[:, :, HS:HS + 1],
                            ones4[:].rearrange("p (kt c) -> p kt c", c=1))
                # new K / V natural layout (knew/vnew outputs + V~ tiles)
                for g in range(2):
                    knp = [psA.tile([128, 512], F32, tag="knp", name="knp") for _ in range(HPC)]
                    vnp = [psA.tile([128, 512], F32, tag="vnp", name="vnp") for _ in range(HPC)]
                    for i in range(8):
                        kt = g * 8 + i
                        ksl = slice(kt * 128, (kt + 1) * 128)
                        osl = slice(i * 64, (i + 1) * 64)
                        for h in range(HPC):
                            hsl = slice(64 * h, 64 * h + 64)
                            nc.tensor.matmul(knp[h][:, osl], XT[b][hsl, ksl],
                                             wk2r[hsl, :], start=True, stop=True,
                                             tile_position=(64 * h, 0))
                            nc.tensor.matmul(vnp[h][:, osl], XT[b][hsl, ksl],
                                             wv2r[hsl, :], start=True, stop=True,
                                             tile_position=(64 * h, 0))
                    for h in range(HPC):
                        kns = work.tile([128, 512], F32, tag="kns")
                        nc.vector.tensor_copy(kns[:], knp[h][:])
                        dst = knew[b, h].rearrange("(kt p) d -> p kt d", p=128)
                        nc.sync.dma_start(
                            out=dst[:, g * 8:(g + 1) * 8, :],
                            in_=kns[:].rearrange("p (kt d) -> p kt d", d=64))
                        # V: evac rounded into V~ strided cols, then DMA out
                        vv = vt_view(b, h)[:, 16 + g * 8:16 + (g + 1) * 8, :]
                        nc.vector.tensor_copy(
                            vv[:, :, 0:HS],
                            vnp[h][:].rearrange("p (kt d) -> p kt d", d=64))
                        nc.vector.tensor_copy(
                            vv[:, :, HS:HS + 1],
                            ones8[:].rearrange("p (kt c) -> p kt c", c=1))
                        vdst = vnew[b, h].rearrange("(kt p) d -> p kt d", p=128)
                        nc.sync.dma_start(
                            out=vdst[:, g * 8:(g + 1) * 8, :],
                            in_=vv[:, :, 0:HS].bitcast(F32))

        # ---------------- Phase B: attention + partial out-proj ----------
        with tc.tile_pool(name="psS", bufs=1, space="PSUM") as psS, \
             tc.tile_pool(name="psAV", bufs=2, space="PSUM") as psAV, \
             tc.tile_pool(name="pp", bufs=3) as ppool, \
             tc.tile_pool(name="np", bufs=2) as npool:
            for b in range(B):
                for qc in range(TN // 512):
                    qsl = slice(qc * 512, (qc + 1) * 512)
                    ktmax = (TP + 512 * (qc + 1)) // 128
                    diag0 = (TP + 512 * qc) // 128
                    av = [psAV.tile([128, 512], F32, tag="avop", name="av")
                          for _ in range(HPC)]
                    kts_groups = [list(range(g, min(g + GRP, ktmax)))
                                  for g in range(0, ktmax, GRP)]
                    for kts in kts_groups:
                        w = len(kts) * 512
                        for h in range(HPC):
                            hsl = slice(64 * h, 64 * h + 64)
                            sp = psS.tile([128, GRP * 512], F32, tag=f"sc{h}")
                            for i, kt in enumerate(kts):
                                nc.tensor.matmul(
                                    sp[:, i * 512:(i + 1) * 512],
                                    KT[b][hsl, kt * 128:(kt + 1) * 128],
                                    QT[b][hsl, qsl], start=True, stop=True,
                                    tile_position=(64 * h, 0))
                            pt = ppool.tile([128, GRP * 512], F32R, tag="pt")
                            nc.scalar.activation(pt[:, 0:w], sp[:, 0:w], Exp,
                                                 scale=0.125)
                            if b == 0 and qc == 0 and h == 0 and kts[0] == 0:
                                nc.sync.dma_start(out=dbg_pt[:, 0:w],
                                                  in_=pt[:, 0:w].bitcast(F32))
                            for i, kt in enumerate(kts):
                                psl = slice(i * 512, (i + 1) * 512)
                                j = kt - diag0
                                if j >= 0:
                                    off = 384 - 128 * j
                                    nc.vector.tensor_tensor(
                                        pt[:, psl], pt[:, psl],
                                        caus[:, off:off + 512], MUL)
                                nc.tensor.matmul(
                                    av[h][0:65, :], vt_view(b, h)[:, kt, :],
                                    pt[:, psl], start=(kt == 0),
                                    stop=(kt == ktmax - 1),
                                    skip_group_check=True)
                    # stream end: softmax denominators + normalize
                    dp = npool.tile([128, 512], F32, tag="dp")
                    nc.gpsimd.memset(dp[:], 1.0)
                    nc.vector.tensor_copy(dp[0:1, :], av[0][64:65, :])
                    nc.vector.tensor_copy(dp[64:65, :], av[1][64:65, :])
                    oh = npool.tile([128, 512], F32, tag="oh")
                    nc.vector.tensor_copy(oh[0:64, :], av[0][0:64, :])
                    nc.vector.tensor_copy(oh[64:128, :], av[1][0:64, :])
                    rc = npool.tile([128, 512], F32, tag="rc")
                    nc.vector.reciprocal_approx_fast(out=rc[:], in_=dp[:])
                    t64 = npool.tile([1, 512], F32, tag="t64")
                    nc.vector.tensor_copy(t64[:], rc[64:65, :])
                    rb = npool.tile([128, 512], F32, tag="rb")
                    nc.gpsimd.partition_broadcast(rb[:], rc[0:1, :])
                    rb2 = npool.tile([128, 512], F32, tag="rb2")
                    nc.gpsimd.partition_broadcast(rb2[:], t64[0:1, :])
                    ohn = npool.tile([128, 512], F32R, tag="ohn")
                    nc.vector.tensor_tensor(ohn[0:64, :], oh[0:64, :],
                                            rb[0:64, :], MUL)
                    nc.vector.tensor_tensor(ohn[64:128, :], oh[64:128, :],
                                            rb2[64:128, :], MUL)
                    if b == 0 and qc == 0:
                        nc.sync.dma_start(out=dbg_kt[:], in_=KT[0][:].bitcast(F32))
                        nc.sync.dma_start(out=dbg_qt[:], in_=QT[0][:].bitcast(F32))
                        nc.sync.dma_start(out=dbg_dp[:], in_=dp[:])
                        nc.sync.dma_start(out=dbg_oh[:], in_=oh[:])
                        nc.sync.dma_start(out=dbg_rb[:], in_=rb[:])
                        nc.sync.dma_start(out=dbg_ohn[:], in_=ohn[:].bitcast(F32))
                    # partial out-projection for these 512 rows
                    for r in range(4):
                        for m in range(2):
                            op = psAV.tile([128, 512], F32, tag="avop")
                            nc.tensor.matmul(op[:],
                                             ohn[:, r * 128:(r + 1) * 128],
                                             wor[:, m * 512:(m + 1) * 512],
                                             start=True, stop=True)
                            po = npool.tile([128, 512], F32, tag="po")
                            nc.vector.tensor_copy(po[:], op[:])
                            row0 = b * TN + qc * 512 + r * 128
                            nc.sync.dma_start(
                                out=pout[row0:row0 + 128,
                                         m * 512:(m + 1) * 512],
                                in_=po[:])
    nc.compile()
    return nc


def _get_program():
    if "nc" not in _cache:
        _cache["nc"] = _build_program()
    return _cache["nc"]


def _numpy_fallback(x, pad_mask, past_k, past_v, Wq, Wk, Wv, Wo, bo):
    xh = x.reshape(B, TN, H, HS)
    q = np.einsum("bthd,ed->bhte", xh, Wq, optimize=True)
    k_new = np.einsum("bthd,ed->bhte", xh, Wk, optimize=True)
    v_new = np.einsum("bthd,ed->bhte", xh, Wv, optimize=True)
    k = np.concatenate([past_k, k_new], axis=2)
    v = np.concatenate([past_v, v_new], axis=2)
    scale = 1.0 / np.sqrt(HS)
    scores = np.einsum("bhqd,bhkd->bhqk", q, k, optimize=True) * scale
    causal = np.arange(TT)[None, :] <= (np.arange(TN)[:, None] + TP)
    mask = causal[None, None] & pad_mask[:, None, None, :]
    scores = np.where(mask, scores, np.float32(-1e30))
    scores -= scores.max(axis=-1, keepdims=True)
    e = np.exp(scores)
    attn = e / e.sum(axis=-1, keepdims=True)
    out = np.einsum("bhqk,bhkd->bhqd", attn, v, optimize=True)
    out = out.transpose(0, 2, 1, 3).reshape(B, TN, D)
    out = out @ Wo.T + bo
    return (out.astype(np.float32), k.astype(np.float32),
            v.astype(np.float32))


def kernel(x, pad_mask, past_k, past_v, Wq, Wk, Wv, Wo, bo):
    x = np.ascontiguousarray(np.asarray(x, dtype=np.float32))
    pad_mask = np.asarray(pad_mask)
    past_k = np.ascontiguousarray(np.asarray(past_k, dtype=np.float32))
    past_v = np.ascontiguousarray(np.asarray(past_v, dtype=np.float32))
    Wq = np.asarray(Wq, dtype=np.float32)
    Wk = np.asarray(Wk, dtype=np.float32)
    Wv = np.asarray(Wv, dtype=np.float32)
    Wo = np.asarray(Wo, dtype=np.float32)
    bo = np.asarray(bo, dtype=np.float32)

    if not bool(pad_mask.all()):
        return _numpy_fallback(x, pad_mask.astype(bool), past_k, past_v,
                               Wq, Wk, Wv, Wo, bo)

    from concourse.bass_utils import run_bass_kernel_spmd

    nc = _get_program()

    # host-side shared prep
    causal = (np.arange(128)[:, None] <= np.arange(896)[None, :] - 384
              ).astype(np.float32)
    wk2 = np.ascontiguousarray(np.vstack([Wk.T, Wk.T]))
    wq2 = np.ascontiguousarray(np.vstack([Wq.T, Wq.T]))
    wv2 = np.ascontiguousarray(np.vstack([Wv.T, Wv.T]))
    xf = x.reshape(B * TN, D)

    in_maps = []
    for c in range(NCORES):
        cs = slice(c * 128, (c + 1) * 128)
        hs = slice(HPC * c, HPC * (c + 1))
        in_maps.append({
            "xh": np.ascontiguousarray(xf[:, cs]),
            "pk": np.ascontiguousarray(past_k[:, hs]),
            "pv": np.ascontiguousarray(past_v[:, hs]),
            "wk2": wk2, "wq2": wq2, "wv2": wv2,
            "wo": np.ascontiguousarray(Wo[:, cs].T),
            "causal": causal,
        })

    _cache["last_in_maps"] = in_maps
    trace = bool(int(os.environ.get("KERNEL_PROFILE", "0")))
    res = run_bass_kernel_spmd(nc, in_maps, list(range(NCORES)), trace=trace)
    _cache["last_exec_time_ns"] = res.exec_time_ns

    out = np.zeros((B * TN, D), dtype=np.float32)
    for c in range(NCORES):
        out += res.results[c]["pout"]
    out += bo[None, :]
    out = out.reshape(B, TN, D)

    k = np.empty((B, H, TT, HS), dtype=np.float32)
    v = np.empty((B, H, TT, HS), dtype=np.float32)
    k[:, :, :TP] = past_k
    v[:, :, :TP] = past_v
    for c in range(NCORES):
        hs = slice(HPC * c, HPC * (c + 1))
        k[:, hs, TP:] = res.results[c]["knew"]
        v[:, hs, TP:] = res.results[c]["vnew"]
    return out, k, v


# revision 12
# speedup vs baseline: 1.7440x; 1.7440x over previous
"""MultiHeadAttentionDecoder kernel for 8 Trainium2 NeuronCores.

Sharding: 8-way tensor parallel over heads (2 heads per core), both batches
and all queries on every core.  Each core:
  - projects Q/K/V for its 2 heads (transposed layouts via PE transpose),
  - runs causal flash-attention (scores kept transposed: [keys, queries]),
    softmax without max-subtraction (scores are O(1) bounded), denominator
    via a ones-column appended to V,
  - computes its partial output projection (row-shard of W_out).
Host sums the 8 partial projections (+bias) and concatenates past K/V with
the new K/V computed on device.  Big matmuls run as float32r (~1e-4 rel).
"""

import os
import sys

import numpy as np

sys.path.insert(0, "/opt/trn_rl_repo")

B, H, TN, TP, HS, D = 2, 16, 2048, 2048, 64, 1024
TT = TN + TP                     # 4096 total keys
NCORES = 8
HPC = H // NCORES                # 2 heads per core
KT_N = TT // 128                 # 32 key tiles of 128
GRP = 3                          # score k-tiles per exp group

_cache = {}


def _build_program():
    import concourse.mybir as mybir
    from concourse import bacc
    from concourse.masks import make_identity
    from concourse.tile import TileContext

    F32 = mybir.dt.float32
    F32R = mybir.dt.float32r
    Exp = mybir.ActivationFunctionType.Exp
    MUL = mybir.AluOpType.mult

    nc = bacc.Bacc("TRN2", target_bir_lowering=False, debug=False,
                   num_devices=NCORES)

    xh = nc.dram_tensor("xh", [B * TN, 128], F32, kind="ExternalInput")
    pk = nc.dram_tensor("pk", [B, HPC, TP, HS], F32, kind="ExternalInput")
    pv = nc.dram_tensor("pv", [B, HPC, TP, HS], F32, kind="ExternalInput")
    wk2 = nc.dram_tensor("wk2", [128, HS], F32, kind="ExternalInput")
    wq2 = nc.dram_tensor("wq2", [128, HS], F32, kind="ExternalInput")
    wv2 = nc.dram_tensor("wv2", [128, HS], F32, kind="ExternalInput")
    wo = nc.dram_tensor("wo", [128, D], F32, kind="ExternalInput")
    causal = nc.dram_tensor("causal", [128, 896], F32, kind="ExternalInput")
    pout = nc.dram_tensor("pout", [B * TN, D], F32, kind="ExternalOutput")
    knew = nc.dram_tensor("knew", [B, HPC, TN, HS], F32, kind="ExternalOutput")
    vnew = nc.dram_tensor("vnew", [B, HPC, TN, HS], F32, kind="ExternalOutput")

    from contextlib import ExitStack
    with TileContext(nc) as tc, ExitStack() as ctx:
        perm = ctx.enter_context(tc.tile_pool(name="perm", bufs=1))
        work = ctx.enter_context(tc.tile_pool(name="work", bufs=3))
        # --- constants ---
        ident = perm.tile([128, 128], F32, tag="ident")
        make_identity(nc, ident[:])
        wk2r = perm.tile([128, HS], F32R, tag="wk2r")
        wq2r = perm.tile([128, HS], F32R, tag="wq2r")
        wv2r = perm.tile([128, HS], F32R, tag="wv2r")
        wor = perm.tile([128, D], F32R, tag="wor")
        caus = perm.tile([128, 896], F32R, tag="caus")
        nc.gpsimd.dma_start(out=wk2r[:], in_=wk2[:])
        nc.gpsimd.dma_start(out=wq2r[:], in_=wq2[:])
        nc.gpsimd.dma_start(out=wv2r[:], in_=wv2[:])
        nc.gpsimd.dma_start(out=wor[:], in_=wo[:])
        nc.gpsimd.dma_start(out=caus[:], in_=causal[:])
        ones4 = perm.tile([128, 4], F32, tag="ones4")
        nc.gpsimd.memset(ones4[:], 1.0)
        ones8 = perm.tile([128, 8], F32, tag="ones8")
        nc.gpsimd.memset(ones8[:], 1.0)

        # --- persistent activations ---
        XT = [perm.tile([128, TN], F32R, tag=f"XT{b}", name=f"XT{b}") for b in range(B)]
        KT = [perm.tile([128, TT], F32R, tag=f"KT{b}", name=f"KT{b}") for b in range(B)]
        QT = [perm.tile([128, TN], F32R, tag=f"QT{b}", name=f"QT{b}") for b in range(B)]
        VT = [[perm.tile([128, KT_N * (HS + 1)], F32R, tag=f"VT{b}{h}", name=f"VT{b}{h}")
               for h in range(HPC)] for b in range(B)]

        def vt_view(b, h):
            return VT[b][h][:].rearrange("p (kt c) -> p kt c", c=HS + 1)

        # ---------------- Phase A: projections / layouts ----------------
        with tc.tile_pool(name="psA", bufs=2, space="PSUM") as psA:
            for b in range(B):
                # X^T for this core's 128 embed dims (2 heads x 64)
                xv = xh[b * TN:(b + 1) * TN, :].rearrange(
                    "(t p) e -> p t e", p=128)
                for tg in range(4):
                    xsg = work.tile([128, 4, 128], F32, tag="xsg")
                    nc.sync.dma_start(out=xsg[:], in_=xv[:, tg * 4:(tg + 1) * 4, :])
                    for i in range(4):
                        t = tg * 4 + i
                        tp = psA.tile([128, 128], F32, tag="tp")
                        nc.tensor.transpose(tp[:], xsg[:, i, :], ident[:])
                        nc.vector.tensor_copy(XT[b][:, t * 128:(t + 1) * 128],
                                              tp[:])
                # past K^T (head pair stacked on partitions)
                for tg in range(4):
                    ksg = work.tile([128, 4, 128], F32, tag="ksg")
                    for h in range(HPC):
                        src = pk[b, h, tg * 512:(tg + 1) * 512, :].rearrange(
                            "(kt p) d -> p kt d", p=128)
                        nc.sync.dma_start(out=ksg[:, :, 64 * h:64 * h + 64],
                                          in_=src)
                    for i in range(4):
                        t = tg * 4 + i
                        tp = psA.tile([128, 128], F32, tag="tp")
                        nc.tensor.transpose(tp[:], ksg[:, i, :], ident[:])
                        nc.vector.tensor_copy(KT[b][:, t * 128:(t + 1) * 128],
                                              tp[:])
                # new K^T and Q^T (col-tiled head pair)
                for kc in range(TN // 512):
                    sl = slice(kc * 512, (kc + 1) * 512)
                    nsl = slice(TP + kc * 512, TP + (kc + 1) * 512)
                    kp0 = psA.tile([64, 512], F32, tag="kqp", name="kp0")
                    kp1 = psA.tile([64, 512], F32, tag="kqp", name="kp1")
                    nc.tensor.matmul(kp0[:], wk2r[0:64, :], XT[b][0:64, sl],
                                     start=True, stop=True, tile_position=(0, 0))
                    nc.tensor.matmul(kp1[:], wk2r[64:128, :],
                                     XT[b][64:128, sl], start=True, stop=True,
                                     tile_position=(64, 0))
                    nc.vector.tensor_copy(KT[b][0:64, nsl], kp0[:])
                    nc.vector.tensor_copy(KT[b][64:128, nsl], kp1[:])
                    qp0 = psA.tile([64, 512], F32, tag="kqp", name="qp0")
                    qp1 = psA.tile([64, 512], F32, tag="kqp", name="qp1")
                    nc.tensor.matmul(qp0[:], wq2r[0:64, :], XT[b][0:64, sl],
                                     start=True, stop=True, tile_position=(0, 0))
                    nc.tensor.matmul(qp1[:], wq2r[64:128, :],
                                     XT[b][64:128, sl], start=True, stop=True,
                                     tile_position=(64, 0))
                    nc.vector.tensor_copy(QT[b][0:64, sl], qp0[:])
                    nc.vector.tensor_copy(QT[b][64:128, sl], qp1[:])
                # past V into V~ tiles (fp32r cast during SWDGE DMA) + ones col
                for h in range(HPC):
                    for g in range(4):
                        vv = vt_view(b, h)[:, g * 4:(g + 1) * 4, :]
                        src = pv[b, h, g * 512:(g + 1) * 512, :].rearrange(
                            "(kt p) d -> p kt d", p=128)
                        nc.gpsimd.dma_start(out=vv[:, :, 0:HS], in_=src)
                        nc.vector.tensor_copy(
                            vv[:, :, HS:HS + 1],
                            ones4[:].rearrange("p (kt c) -> p kt c", c=1))
                # new K / V natural layout (knew/vnew outputs + V~ tiles)
                for g in range(2):
                    knp = [psA.tile([128, 512], F32, tag="knp", name="knp") for _ in range(HPC)]
                    vnp = [psA.tile([128, 512], F32, tag="vnp", name="vnp") for _ in range(HPC)]
                    for i in range(8):
                        kt = g * 8 + i
                        ksl = slice(kt * 128, (kt + 1) * 128)
                        osl = slice(i * 64, (i + 1) * 64)
                        for h in range(HPC):
                            hsl = slice(64 * h, 64 * h + 64)
                            nc.tensor.matmul(knp[h][:, osl], XT[b][hsl, ksl],
                                             wk2r[hsl, :], start=True, stop=True,
                                             tile_position=(64 * h, 0))
                            nc.tensor.matmul(vnp[h][:, osl], XT[b][hsl, ksl],
                                             wv2r[hsl, :], start=True, stop=True,
                                             tile_position=(64 * h, 0))
                    for h in range(HPC):
                        kns = work.tile([128, 512], F32, tag="kns")
                        nc.vector.tensor_copy(kns[:], knp[h][:])
                        dst = knew[b, h].rearrange("(kt p) d -> p kt d", p=128)
                        nc.sync.dma_start(
                            out=dst[:, g * 8:(g + 1) * 8, :],
                            in_=kns[:].rearrange("p (kt d) -> p kt d", d=64))
                        # V: evac rounded into V~ strided cols, then DMA out
                        vv = vt_view(b, h)[:, 16 + g * 8:16 + (g + 1) * 8, :]
                        nc.vector.tensor_copy(
                            vv[:, :, 0:HS],
                            vnp[h][:].rearrange("p (kt d) -> p kt d", d=64))
                        nc.vector.tensor_copy(
                            vv[:, :, HS:HS + 1],
                            ones8[:].rearrange("p (kt c) -> p kt c", c=1))
                        vdst = vnew[b, h].rearrange("(kt p) d -> p kt d", p=128)
                        nc.sync.dma_start(
                            out=vdst[:, g * 8:(g + 1) * 8, :],
                            in_=vv[:, :, 0:HS].bitcast(F32))

        # ---------------- Phase B: attention + partial out-proj ----------
        # Flat (head, ktile) score units packed into 3-bank psum groups,
        # double-buffered so exp (ACT) streams continuously.  Each stream's
        # normalization runs at its end; its 8 out-projection matmuls are
        # deferred and interleaved into the next stream's groups so the PE
        # FIFO never stalls the ACT pipeline.
        with tc.tile_pool(name="psS", bufs=2, space="PSUM") as psS, \
             tc.tile_pool(name="psAV", bufs=2, space="PSUM") as psAV, \
             tc.tile_pool(name="pp", bufs=3) as ppool, \
             tc.tile_pool(name="np", bufs=2) as npool:
            pending = []        # deferred out-proj emitters from prev stream

            def emit_pending(n):
                for _ in range(min(n, len(pending))):
                    pending.pop(0)()

            for b in range(B):
                for qc in range(TN // 512):
                    qsl = slice(qc * 512, (qc + 1) * 512)
                    ktmax = (TP + 512 * (qc + 1)) // 128
                    diag0 = (TP + 512 * qc) // 128
                    # flat (h, kt) units, heads interleaved for PE pairing
                    units = []
                    for kt in range(ktmax):
                        for h in range(HPC):
                            units.append((h, kt))
                    av = [None, None]
                    for g0 in range(0, len(units), GRP):
                        grp = units[g0:g0 + GRP]
                        w = len(grp) * 512
                        sp = psS.tile([128, GRP * 512], F32, tag="sc")
                        for i, (h, kt) in enumerate(grp):
                            hsl = slice(64 * h, 64 * h + 64)
                            nc.tensor.matmul(
                                sp[:, i * 512:(i + 1) * 512],
                                KT[b][hsl, kt * 128:(kt + 1) * 128],
                                QT[b][hsl, qsl], start=True, stop=True,
                                tile_position=(64 * h, 0))
                        pt = ppool.tile([128, GRP * 512], F32R, tag="pt")
                        nc.scalar.activation(pt[:, 0:w], sp[:, 0:w], Exp,
                                             scale=0.125)
                        for i, (h, kt) in enumerate(grp):
                            psl = slice(i * 512, (i + 1) * 512)
                            j = kt - diag0
                            if j >= 0:
                                off = 384 - 128 * j
                                nc.vector.tensor_tensor(
                                    pt[:, psl], pt[:, psl],
                                    caus[:, off:off + 512], MUL)
                            if av[h] is None:
                                av[h] = psAV.tile([128, 512], F32, tag="avop",
                                                  name="av")
                            nc.tensor.matmul(
                                av[h][0:65, :], vt_view(b, h)[:, kt, :],
                                pt[:, psl], start=(kt == 0),
                                stop=(kt == ktmax - 1),
                                skip_group_check=True)
                        emit_pending(2)
                    emit_pending(8)
                    # stream end: denominators + normalize (DVE/GPS work that
                    # runs under the next stream's exps)
                    dp = npool.tile([128, 512], F32, tag="dp")
                    nc.gpsimd.memset(dp[:], 1.0)
                    nc.vector.tensor_copy(dp[0:1, :], av[0][64:65, :])
                    nc.vector.tensor_copy(dp[64:65, :], av[1][64:65, :])
                    oh = npool.tile([128, 512], F32, tag="oh")
                    nc.vector.tensor_copy(oh[0:64, :], av[0][0:64, :])
                    nc.vector.tensor_copy(oh[64:128, :], av[1][0:64, :])
                    rc = npool.tile([128, 512], F32, tag="rc")
                    nc.vector.reciprocal_approx_fast(out=rc[:], in_=dp[:])
                    t64 = npool.tile([1, 512], F32, tag="t64")
                    nc.vector.tensor_copy(t64[:], rc[64:65, :])
                    rb = npool.tile([128, 512], F32, tag="rb")
                    nc.gpsimd.partition_broadcast(rb[:], rc[0:1, :])
                    rb2 = npool.tile([128, 512], F32, tag="rb2")
                    nc.gpsimd.partition_broadcast(rb2[:], t64[0:1, :])
                    ohn = npool.tile([128, 512], F32R, tag="ohn")
                    nc.vector.tensor_tensor(ohn[0:64, :], oh[0:64, :],
                                            rb[0:64, :], MUL)
                    nc.vector.tensor_tensor(ohn[64:128, :], oh[64:128, :],
                                            rb2[64:128, :], MUL)

                    def make_op(b_, qc_, r_, m_, ohn_):
                        def emit():
                            op = psAV.tile([128, 512], F32, tag="avop",
                                           name="op")
                            nc.tensor.matmul(op[:],
                                             ohn_[:, r_ * 128:(r_ + 1) * 128],
                                             wor[:, m_ * 512:(m_ + 1) * 512],
                                             start=True, stop=True)
                            po = npool.tile([128, 512], F32, tag="po")
                            nc.vector.tensor_copy(po[:], op[:])
                            row0 = b_ * TN + qc_ * 512 + r_ * 128
                            nc.sync.dma_start(
                                out=pout[row0:row0 + 128,
                                         m_ * 512:(m_ + 1) * 512],
                                in_=po[:])
                        return emit

                    for r in range(4):
                        for m in range(2):
                            pending.append(make_op(b, qc, r, m, ohn))
            emit_pending(8)
    nc.compile()
    return nc


def _get_program():
    if "nc" not in _cache:
        _cache["nc"] = _build_program()
    return _cache["nc"]


def _numpy_fallback(x, pad_mask, past_k, past_v, Wq, Wk, Wv, Wo, bo):
    xh = x.reshape(B, TN, H, HS)
    q = np.einsum("bthd,ed->bhte", xh, Wq, optimize=True)
    k_new = np.einsum("bthd,ed->bhte", xh, Wk, optimize=True)
    v_new = np.einsum("bthd,ed->bhte", xh, Wv, optimize=True)
    k = np.concatenate([past_k, k_new], axis=2)
    v = np.concatenate([past_v, v_new], axis=2)
    scale = 1.0 / np.sqrt(HS)
    scores = np.einsum("bhqd,bhkd->bhqk", q, k, optimize=True) * scale
    causal = np.arange(TT)[None, :] <= (np.arange(TN)[:, None] + TP)
    mask = causal[None, None] & pad_mask[:, None, None, :]
    scores = np.where(mask, scores, np.float32(-1e30))
    scores -= scores.max(axis=-1, keepdims=True)
    e = np.exp(scores)
    attn = e / e.sum(axis=-1, keepdims=True)
    out = np.einsum("bhqk,bhkd->bhqd", attn, v, optimize=True)
    out = out.transpose(0, 2, 1, 3).reshape(B, TN, D)
    out = out @ Wo.T + bo
    return (out.astype(np.float32), k.astype(np.float32),
            v.astype(np.float32))


def kernel(x, pad_mask, past_k, past_v, Wq, Wk, Wv, Wo, bo):
    x = np.ascontiguousarray(np.asarray(x, dtype=np.float32))
    pad_mask = np.asarray(pad_mask)
    past_k = np.ascontiguousarray(np.asarray(past_k, dtype=np.float32))
    past_v = np.ascontiguousarray(np.asarray(past_v, dtype=np.float32))
    Wq = np.asarray(Wq, dtype=np.float32)
    Wk = np.asarray(Wk, dtype=np.float32)
    Wv = np.asarray(Wv, dtype=np.float32)
    Wo = np.asarray(Wo, dtype=np.float32)
    bo = np.asarray(bo, dtype=np.float32)

    if not bool(pad_mask.all()):
        return _numpy_fallback(x, pad_mask.astype(bool), past_k, past_v,
                               Wq, Wk, Wv, Wo, bo)

    from concourse.bass_utils import run_bass_kernel_spmd

    nc = _get_program()

    # host-side shared prep
    causal = (np.arange(128)[:, None] <= np.arange(896)[None, :] - 384
              ).astype(np.float32)
    wk2 = np.ascontiguousarray(np.vstack([Wk.T, Wk.T]))
    wq2 = np.ascontiguousarray(np.vstack([Wq.T, Wq.T]))
    wv2 = np.ascontiguousarray(np.vstack([Wv.T, Wv.T]))
    xf = x.reshape(B * TN, D)

    in_maps = []
    for c in range(NCORES):
        cs = slice(c * 128, (c + 1) * 128)
        hs = slice(HPC * c, HPC * (c + 1))
        in_maps.append({
            "xh": np.ascontiguousarray(xf[:, cs]),
            "pk": np.ascontiguousarray(past_k[:, hs]),
            "pv": np.ascontiguousarray(past_v[:, hs]),
            "wk2": wk2, "wq2": wq2, "wv2": wv2,
            "wo": np.ascontiguousarray(Wo[:, cs].T),
            "causal": causal,
        })

    _cache["last_in_maps"] = in_maps
    trace = bool(int(os.environ.get("KERNEL_PROFILE", "0")))
    res = run_bass_kernel_spmd(nc, in_maps, list(range(NCORES)), trace=trace)
    _cache["last_exec_time_ns"] = res.exec_time_ns

    out = np.zeros((B * TN, D), dtype=np.float32)
    for c in range(NCORES):
        out += res.results[c]["pout"]
    out += bo[None, :]
    out = out.reshape(B, TN, D)

    k = np.empty((B, H, TT, HS), dtype=np.float32)
    v = np.empty((B, H, TT, HS), dtype=np.float32)
    k[:, :, :TP] = past_k
    v[:, :, :TP] = past_v
    for c in range(NCORES):
        hs = slice(HPC * c, HPC * (c + 1))
        k[:, hs, TP:] = res.results[c]["knew"]
        v[:, hs, TP:] = res.results[c]["vnew"]
    return out, k, v


# revision 14
# speedup vs baseline: 13.3585x; 7.6597x over previous
"""MultiHeadAttentionDecoder kernel for 8 Trainium2 NeuronCores.

Sharding: 8-way tensor parallel over heads (2 heads per core), both batches
and all queries on every core.  Each core:
  - projects Q/K/V for its 2 heads (transposed layouts via PE transpose),
  - runs causal flash-attention (scores kept transposed: [keys, queries]),
    softmax without max-subtraction (scores are O(1) bounded), denominator
    via a ones-column appended to V,
  - computes its partial output projection (row-shard of W_out).
Host sums the 8 partial projections (+bias) and concatenates past K/V with
the new K/V computed on device.  Big matmuls run as float32r (~1e-4 rel).
"""

import os
import sys

import numpy as np

sys.path.insert(0, "/opt/trn_rl_repo")

B, H, TN, TP, HS, D = 2, 16, 2048, 2048, 64, 1024
TT = TN + TP                     # 4096 total keys
NCORES = 8
HPC = H // NCORES                # 2 heads per core
KT_N = TT // 128                 # 32 key tiles of 128
GRP = 2                          # score units per exp group

_cache = {}


def _build_program():
    import concourse.mybir as mybir
    from concourse import bacc
    from concourse.masks import make_identity
    from concourse.tile import TileContext

    F32 = mybir.dt.float32
    F32R = mybir.dt.float32r
    Exp = mybir.ActivationFunctionType.Exp
    MUL = mybir.AluOpType.mult

    nc = bacc.Bacc("TRN2", target_bir_lowering=False, debug=False,
                   num_devices=NCORES)

    xh = nc.dram_tensor("xh", [B * TN, 128], F32, kind="ExternalInput")
    pk = nc.dram_tensor("pk", [B, HPC, TP, HS], F32, kind="ExternalInput")
    pv = nc.dram_tensor("pv", [B, HPC, TP, HS], F32, kind="ExternalInput")
    wk2 = nc.dram_tensor("wk2", [128, HS], F32, kind="ExternalInput")
    wq2 = nc.dram_tensor("wq2", [128, HS], F32, kind="ExternalInput")
    wv2 = nc.dram_tensor("wv2", [128, HS], F32, kind="ExternalInput")
    wo = nc.dram_tensor("wo", [128, D], F32, kind="ExternalInput")
    causal = nc.dram_tensor("causal", [128, 896], F32, kind="ExternalInput")
    pout = nc.dram_tensor("pout", [B * TN, D], F32, kind="ExternalOutput")
    knew = nc.dram_tensor("knew", [B, HPC, TN, HS], F32, kind="ExternalOutput")
    vnew = nc.dram_tensor("vnew", [B, HPC, TN, HS], F32, kind="ExternalOutput")

    from contextlib import ExitStack
    with TileContext(nc) as tc, ExitStack() as ctx:
        perm = ctx.enter_context(tc.tile_pool(name="perm", bufs=1))
        work = ctx.enter_context(tc.tile_pool(name="work", bufs=3))
        # --- constants ---
        ident = perm.tile([128, 128], F32, tag="ident")
        make_identity(nc, ident[:])
        wk2r = perm.tile([128, HS], F32R, tag="wk2r")
        wq2r = perm.tile([128, HS], F32R, tag="wq2r")
        wv2r = perm.tile([128, HS], F32R, tag="wv2r")
        wor = perm.tile([128, D], F32R, tag="wor")
        caus = perm.tile([128, 896], F32R, tag="caus")
        nc.gpsimd.dma_start(out=wk2r[:], in_=wk2[:])
        nc.gpsimd.dma_start(out=wq2r[:], in_=wq2[:])
        nc.gpsimd.dma_start(out=wv2r[:], in_=wv2[:])
        nc.gpsimd.dma_start(out=wor[:], in_=wo[:])
        nc.gpsimd.dma_start(out=caus[:], in_=causal[:])
        ones4 = perm.tile([128, 4], F32, tag="ones4")
        nc.gpsimd.memset(ones4[:], 1.0)
        ones8 = perm.tile([128, 8], F32, tag="ones8")
        nc.gpsimd.memset(ones8[:], 1.0)

        # --- persistent activations ---
        XT = [perm.tile([128, TN], F32R, tag=f"XT{b}", name=f"XT{b}") for b in range(B)]
        KT = [perm.tile([128, TT], F32R, tag=f"KT{b}", name=f"KT{b}") for b in range(B)]
        QT = [perm.tile([128, TN], F32R, tag=f"QT{b}", name=f"QT{b}") for b in range(B)]
        VT = [[perm.tile([128, KT_N * (HS + 1)], F32R, tag=f"VT{b}{h}", name=f"VT{b}{h}")
               for h in range(HPC)] for b in range(B)]

        def vt_view(b, h):
            return VT[b][h][:].rearrange("p (kt c) -> p kt c", c=HS + 1)

        # ---------------- Phase A: projections / layouts ----------------
        with tc.tile_pool(name="psA", bufs=2, space="PSUM") as psA:
            for b in range(B):
                # X^T for this core's 128 embed dims (2 heads x 64)
                xv = xh[b * TN:(b + 1) * TN, :].rearrange(
                    "(t p) e -> p t e", p=128)
                for tg in range(4):
                    xsg = work.tile([128, 4, 128], F32, tag="xsg")
                    nc.sync.dma_start(out=xsg[:], in_=xv[:, tg * 4:(tg + 1) * 4, :])
                    for i in range(4):
                        t = tg * 4 + i
                        tp = psA.tile([128, 128], F32, tag="tp")
                        nc.tensor.transpose(tp[:], xsg[:, i, :], ident[:])
                        nc.vector.tensor_copy(XT[b][:, t * 128:(t + 1) * 128],
                                              tp[:])
                # past K^T (head pair stacked on partitions)
                for tg in range(4):
                    ksg = work.tile([128, 4, 128], F32, tag="ksg")
                    for h in range(HPC):
                        src = pk[b, h, tg * 512:(tg + 1) * 512, :].rearrange(
                            "(kt p) d -> p kt d", p=128)
                        nc.sync.dma_start(out=ksg[:, :, 64 * h:64 * h + 64],
                                          in_=src)
                    for i in range(4):
                        t = tg * 4 + i
                        tp = psA.tile([128, 128], F32, tag="tp")
                        nc.tensor.transpose(tp[:], ksg[:, i, :], ident[:])
                        nc.vector.tensor_copy(KT[b][:, t * 128:(t + 1) * 128],
                                              tp[:])
                # new K^T and Q^T (col-tiled head pair)
                for kc in range(TN // 512):
                    sl = slice(kc * 512, (kc + 1) * 512)
                    nsl = slice(TP + kc * 512, TP + (kc + 1) * 512)
                    kp0 = psA.tile([64, 512], F32, tag="kqp", name="kp0")
                    kp1 = psA.tile([64, 512], F32, tag="kqp", name="kp1")
                    nc.tensor.matmul(kp0[:], wk2r[0:64, :], XT[b][0:64, sl],
                                     start=True, stop=True, tile_position=(0, 0))
                    nc.tensor.matmul(kp1[:], wk2r[64:128, :],
                                     XT[b][64:128, sl], start=True, stop=True,
                                     tile_position=(64, 0))
                    nc.vector.tensor_copy(KT[b][0:64, nsl], kp0[:])
                    nc.vector.tensor_copy(KT[b][64:128, nsl], kp1[:])
                    qp0 = psA.tile([64, 512], F32, tag="kqp", name="qp0")
                    qp1 = psA.tile([64, 512], F32, tag="kqp", name="qp1")
                    nc.tensor.matmul(qp0[:], wq2r[0:64, :], XT[b][0:64, sl],
                                     start=True, stop=True, tile_position=(0, 0))
                    nc.tensor.matmul(qp1[:], wq2r[64:128, :],
                                     XT[b][64:128, sl], start=True, stop=True,
                                     tile_position=(64, 0))
                    nc.vector.tensor_copy(QT[b][0:64, sl], qp0[:])
                    nc.vector.tensor_copy(QT[b][64:128, sl], qp1[:])
                # past V into V~ tiles (fp32r cast during SWDGE DMA) + ones col
                for h in range(HPC):
                    for g in range(4):
                        vv = vt_view(b, h)[:, g * 4:(g + 1) * 4, :]
                        src = pv[b, h, g * 512:(g + 1) * 512, :].rearrange(
                            "(kt p) d -> p kt d", p=128)
                        nc.gpsimd.dma_start(out=vv[:, :, 0:HS], in_=src)
                        nc.vector.tensor_copy(
                            vv[:, :, HS:HS + 1],
                            ones4[:].rearrange("p (kt c) -> p kt c", c=1))
                # new K / V natural layout (knew/vnew outputs + V~ tiles)
                for g in range(2):
                    knp = [psA.tile([128, 512], F32, tag="knp", name="knp") for _ in range(HPC)]
                    vnp = [psA.tile([128, 512], F32, tag="vnp", name="vnp") for _ in range(HPC)]
                    for i in range(8):
                        kt = g * 8 + i
                        ksl = slice(kt * 128, (kt + 1) * 128)
                        osl = slice(i * 64, (i + 1) * 64)
                        for h in range(HPC):
                            hsl = slice(64 * h, 64 * h + 64)
                            nc.tensor.matmul(knp[h][:, osl], XT[b][hsl, ksl],
                                             wk2r[hsl, :], start=True, stop=True,
                                             tile_position=(64 * h, 0))
                            nc.tensor.matmul(vnp[h][:, osl], XT[b][hsl, ksl],
                                             wv2r[hsl, :], start=True, stop=True,
                                             tile_position=(64 * h, 0))
                    for h in range(HPC):
                        kns = work.tile([128, 512], F32, tag="kns")
                        nc.vector.tensor_copy(kns[:], knp[h][:])
                        dst = knew[b, h].rearrange("(kt p) d -> p kt d", p=128)
                        nc.sync.dma_start(
                            out=dst[:, g * 8:(g + 1) * 8, :],
                            in_=kns[:].rearrange("p (kt d) -> p kt d", d=64))
                        # V: evac rounded into V~ strided cols, then DMA out
                        vv = vt_view(b, h)[:, 16 + g * 8:16 + (g + 1) * 8, :]
                        nc.vector.tensor_copy(
                            vv[:, :, 0:HS],
                            vnp[h][:].rearrange("p (kt d) -> p kt d", d=64))
                        nc.vector.tensor_copy(
                            vv[:, :, HS:HS + 1],
                            ones8[:].rearrange("p (kt c) -> p kt c", c=1))
                        vdst = vnew[b, h].rearrange("(kt p) d -> p kt d", p=128)
                        nc.sync.dma_start(
                            out=vdst[:, g * 8:(g + 1) * 8, :],
                            in_=vv[:, :, 0:HS].bitcast(F32))

        # ---------------- Phase B: attention + partial out-proj ----------
        # Flat (head, ktile) score units packed into 3-bank psum groups,
        # double-buffered so exp (ACT) streams continuously.  Each stream's
        # normalization runs at its end; its 8 out-projection matmuls are
        # deferred and interleaved into the next stream's groups so the PE
        # FIFO never stalls the ACT pipeline.
        with tc.tile_pool(name="psS", bufs=2, space="PSUM") as psS, \
             tc.tile_pool(name="psAV", bufs=2, space="PSUM") as psAV, \
             tc.tile_pool(name="psOP", bufs=2, space="PSUM") as psOP, \
             tc.tile_pool(name="pp", bufs=4) as ppool, \
             tc.tile_pool(name="np", bufs=2) as npool:
            pending = []        # deferred out-proj emitters from prev stream

            def emit_pending(n):
                for _ in range(min(n, len(pending))):
                    pending.pop(0)()

            for b in range(B):
                for qc in range(TN // 512):
                    qsl = slice(qc * 512, (qc + 1) * 512)
                    ktmax = (TP + 512 * (qc + 1)) // 128
                    diag0 = (TP + 512 * qc) // 128
                    # flat (h, kt) units, heads interleaved for PE pairing
                    units = []
                    for kt in range(ktmax):
                        for h in range(HPC):
                            units.append((h, kt))
                    av = [None, None]
                    for g0 in range(0, len(units), GRP):
                        grp = units[g0:g0 + GRP]
                        w = len(grp) * 512
                        sp = psS.tile([128, GRP * 512], F32, tag="sc")
                        for i, (h, kt) in enumerate(grp):
                            hsl = slice(64 * h, 64 * h + 64)
                            nc.tensor.matmul(
                                sp[:, i * 512:(i + 1) * 512],
                                KT[b][hsl, kt * 128:(kt + 1) * 128],
                                QT[b][hsl, qsl], start=True, stop=True,
                                tile_position=(64 * h, 0))
                        pt = ppool.tile([128, GRP * 512], F32R, tag="pt")
                        nc.scalar.activation(pt[:, 0:w], sp[:, 0:w], Exp,
                                             scale=0.125)
                        for i, (h, kt) in enumerate(grp):
                            psl = slice(i * 512, (i + 1) * 512)
                            j = kt - diag0
                            if j >= 0:
                                off = 384 - 128 * j
                                nc.vector.tensor_tensor(
                                    pt[:, psl], pt[:, psl],
                                    caus[:, off:off + 512], MUL)
                            if av[h] is None:
                                av[h] = psAV.tile([128, 512], F32, tag="avop",
                                                  name="av")
                            nc.tensor.matmul(
                                av[h][0:65, :], vt_view(b, h)[:, kt, :],
                                pt[:, psl], start=(kt == 0),
                                stop=(kt == ktmax - 1),
                                skip_group_check=True)
                        emit_pending(1)
                    emit_pending(8)
                    # stream end: denominators + normalize (DVE/GPS work that
                    # runs under the next stream's exps)
                    dp = npool.tile([128, 512], F32, tag="dp")
                    nc.gpsimd.memset(dp[:], 1.0)
                    nc.vector.tensor_copy(dp[0:1, :], av[0][64:65, :])
                    nc.vector.tensor_copy(dp[64:65, :], av[1][64:65, :])
                    oh = npool.tile([128, 512], F32, tag="oh")
                    nc.vector.tensor_copy(oh[0:64, :], av[0][0:64, :])
                    nc.vector.tensor_copy(oh[64:128, :], av[1][0:64, :])
                    rc = npool.tile([128, 512], F32, tag="rc")
                    nc.vector.reciprocal_approx_fast(out=rc[:], in_=dp[:])
                    t64 = npool.tile([1, 512], F32, tag="t64")
                    nc.vector.tensor_copy(t64[:], rc[64:65, :])
                    rb = npool.tile([128, 512], F32, tag="rb")
                    nc.gpsimd.partition_broadcast(rb[:], rc[0:1, :])
                    rb2 = npool.tile([128, 512], F32, tag="rb2")
                    nc.gpsimd.partition_broadcast(rb2[:], t64[0:1, :])
                    ohn = npool.tile([128, 512], F32R, tag="ohn")
                    nc.vector.tensor_tensor(ohn[0:64, :], oh[0:64, :],
                                            rb[0:64, :], MUL)
                    nc.vector.tensor_tensor(ohn[64:128, :], oh[64:128, :],
                                            rb2[64:128, :], MUL)

                    def make_op(b_, qc_, r_, m_, ohn_):
                        def emit():
                            op = psOP.tile([128, 512], F32, tag="op",
                                           name="op")
                            for q2 in range(2):
                                nc.tensor.matmul(
                                    op[:, q2 * 256:(q2 + 1) * 256],
                                    ohn_[:, r_ * 128:(r_ + 1) * 128],
                                    wor[:, m_ * 512 + q2 * 256:
                                        m_ * 512 + (q2 + 1) * 256],
                                    start=True, stop=True)
                            po = npool.tile([128, 512], F32, tag="po")
                            nc.vector.tensor_copy(po[:], op[:])
                            row0 = b_ * TN + qc_ * 512 + r_ * 128
                            nc.sync.dma_start(
                                out=pout[row0:row0 + 128,
                                         m_ * 512:(m_ + 1) * 512],
                                in_=po[:])
                        return emit

                    for r in range(4):
                        for m in range(2):
                            pending.append(make_op(b, qc, r, m, ohn))
            emit_pending(8)
    nc.compile()
    return nc


def _get_program():
    if "nc" not in _cache:
        _cache["nc"] = _build_program()
    return _cache["nc"]


def _numpy_fallback(x, pad_mask, past_k, past_v, Wq, Wk, Wv, Wo, bo):
    xh = x.reshape(B, TN, H, HS)
    q = np.einsum("bthd,ed->bhte", xh, Wq, optimize=True)
    k_new = np.einsum("bthd,ed->bhte", xh, Wk, optimize=True)
    v_new = np.einsum("bthd,ed->bhte", xh, Wv, optimize=True)
    k = np.concatenate([past_k, k_new], axis=2)
    v = np.concatenate([past_v, v_new], axis=2)
    scale = 1.0 / np.sqrt(HS)
    scores = np.einsum("bhqd,bhkd->bhqk", q, k, optimize=True) * scale
    causal = np.arange(TT)[None, :] <= (np.arange(TN)[:, None] + TP)
    mask = causal[None, None] & pad_mask[:, None, None, :]
    scores = np.where(mask, scores, np.float32(-1e30))
    scores -= scores.max(axis=-1, keepdims=True)
    e = np.exp(scores)
    attn = e / e.sum(axis=-1, keepdims=True)
    out = np.einsum("bhqk,bhkd->bhqd", attn, v, optimize=True)
    out = out.transpose(0, 2, 1, 3).reshape(B, TN, D)
    out = out @ Wo.T + bo
    return (out.astype(np.float32), k.astype(np.float32),
            v.astype(np.float32))


def kernel(x, pad_mask, past_k, past_v, Wq, Wk, Wv, Wo, bo):
    x = np.ascontiguousarray(np.asarray(x, dtype=np.float32))
    pad_mask = np.asarray(pad_mask)
    past_k = np.ascontiguousarray(np.asarray(past_k, dtype=np.float32))
    past_v = np.ascontiguousarray(np.asarray(past_v, dtype=np.float32))
    Wq = np.asarray(Wq, dtype=np.float32)
    Wk = np.asarray(Wk, dtype=np.float32)
    Wv = np.asarray(Wv, dtype=np.float32)
    Wo = np.asarray(Wo, dtype=np.float32)
    bo = np.asarray(bo, dtype=np.float32)

    if not bool(pad_mask.all()):
        return _numpy_fallback(x, pad_mask.astype(bool), past_k, past_v,
                               Wq, Wk, Wv, Wo, bo)

    from concourse.bass_utils import run_bass_kernel_spmd

    nc = _get_program()

    # host-side shared prep
    causal = (np.arange(128)[:, None] <= np.arange(896)[None, :] - 384
              ).astype(np.float32)
    wk2 = np.ascontiguousarray(np.vstack([Wk.T, Wk.T]))
    wq2 = np.ascontiguousarray(np.vstack([Wq.T, Wq.T]))
    wv2 = np.ascontiguousarray(np.vstack([Wv.T, Wv.T]))
    xf = x.reshape(B * TN, D)

    in_maps = []
    for c in range(NCORES):
        cs = slice(c * 128, (c + 1) * 128)
        hs = slice(HPC * c, HPC * (c + 1))
        in_maps.append({
            "xh": np.ascontiguousarray(xf[:, cs]),
            "pk": np.ascontiguousarray(past_k[:, hs]),
            "pv": np.ascontiguousarray(past_v[:, hs]),
            "wk2": wk2, "wq2": wq2, "wv2": wv2,
            "wo": np.ascontiguousarray(Wo[:, cs].T),
            "causal": causal,
        })

    _cache["last_in_maps"] = in_maps
    trace = bool(int(os.environ.get("KERNEL_PROFILE", "0")))
    if trace:
        try:
            import ntff_hook
            trace = ntff_hook.install()
        except Exception:
            trace = False
    try:
        res = run_bass_kernel_spmd(nc, in_maps, list(range(NCORES)),
                                   trace=trace)
    except Exception:
        if not trace:
            raise
        res = run_bass_kernel_spmd(nc, in_maps, list(range(NCORES)))
    _cache["last_exec_time_ns"] = res.exec_time_ns
    _cache["last_results_obj"] = res

    out = np.zeros((B * TN, D), dtype=np.float32)
    for c in range(NCORES):
        out += res.results[c]["pout"]
    out += bo[None, :]
    out = out.reshape(B, TN, D)

    k = np.empty((B, H, TT, HS), dtype=np.float32)
    v = np.empty((B, H, TT, HS), dtype=np.float32)
    k[:, :, :TP] = past_k
    v[:, :, :TP] = past_v
    for c in range(NCORES):
        hs = slice(HPC * c, HPC * (c + 1))
        k[:, hs, TP:] = res.results[c]["knew"]
        v[:, hs, TP:] = res.results[c]["vnew"]
    return out, k, v


# revision 15
# speedup vs baseline: 15.5790x; 1.1662x over previous
"""MultiHeadAttentionDecoder kernel for 8 Trainium2 NeuronCores.

Sharding: 8-way tensor parallel over heads (2 heads per core), both batches
and all queries on every core.  Each core:
  - projects Q/K/V for its 2 heads (transposed layouts via PE transpose),
  - runs causal flash-attention (scores kept transposed: [keys, queries]),
    softmax without max-subtraction (scores are O(1) bounded), denominator
    via a ones-column appended to V,
  - computes its partial output projection (row-shard of W_out).
Host sums the 8 partial projections (+bias) and concatenates past K/V with
the new K/V computed on device.  Big matmuls run as float32r (~1e-4 rel).
"""

import os
import sys

import numpy as np

sys.path.insert(0, "/opt/trn_rl_repo")

B, H, TN, TP, HS, D = 2, 16, 2048, 2048, 64, 1024
TT = TN + TP                     # 4096 total keys
NCORES = 8
HPC = H // NCORES                # 2 heads per core
KT_N = TT // 128                 # 32 key tiles of 128
GRP = 2                          # score units per exp group

_cache = {}


def _patch_ldw_opt():
    from concourse import bass_utils as _bu
    if getattr(_bu, "_ldwopt_patched", False):
        return
    _orig = _bu.run_command

    def _run(argv, **kw):
        argv = ["--enable-ldw-opt=true" if a == "--enable-ldw-opt=false"
                else a for a in argv]
        return _orig(argv, **kw)

    _bu.run_command = _run
    _bu._ldwopt_patched = True


def _build_program():
    import concourse.mybir as mybir
    if os.environ.get("KERNEL_LDWOPT", "0") == "1":
        _patch_ldw_opt()
    from concourse import bacc
    from concourse.masks import make_identity
    from concourse.tile import TileContext

    F32 = mybir.dt.float32
    F32R = mybir.dt.float32r
    Exp = mybir.ActivationFunctionType.Exp
    MUL = mybir.AluOpType.mult

    nc = bacc.Bacc("TRN2", target_bir_lowering=False, debug=False,
                   num_devices=NCORES)

    xh = nc.dram_tensor("xh", [B * TN, 128], F32, kind="ExternalInput")
    pk = nc.dram_tensor("pk", [B, HPC, TP, HS], F32, kind="ExternalInput")
    pv = nc.dram_tensor("pv", [B, HPC, TP, HS], F32, kind="ExternalInput")
    wk2 = nc.dram_tensor("wk2", [128, HS], F32, kind="ExternalInput")
    wq2 = nc.dram_tensor("wq2", [128, HS], F32, kind="ExternalInput")
    wv2 = nc.dram_tensor("wv2", [128, HS], F32, kind="ExternalInput")
    wo = nc.dram_tensor("wo", [128, D], F32, kind="ExternalInput")
    causal = nc.dram_tensor("causal", [128, 896], F32, kind="ExternalInput")
    pout = nc.dram_tensor("pout", [B * TN, D], F32, kind="ExternalOutput")
    knew = nc.dram_tensor("knew", [B, HPC, TN, HS], F32, kind="ExternalOutput")
    vnew = nc.dram_tensor("vnew", [B, HPC, TN, HS], F32, kind="ExternalOutput")

    from contextlib import ExitStack
    with TileContext(nc) as tc, ExitStack() as ctx:
        perm = ctx.enter_context(tc.tile_pool(name="perm", bufs=1))
        work = ctx.enter_context(tc.tile_pool(name="work", bufs=3))
        # --- constants ---
        ident = perm.tile([128, 128], F32, tag="ident")
        make_identity(nc, ident[:])
        wk2r = perm.tile([128, HS], F32R, tag="wk2r")
        wq2r = perm.tile([128, HS], F32R, tag="wq2r")
        wv2r = perm.tile([128, HS], F32R, tag="wv2r")
        wor = perm.tile([128, D], F32R, tag="wor")
        caus = perm.tile([128, 896], F32R, tag="caus")
        nc.gpsimd.dma_start(out=wk2r[:], in_=wk2[:])
        nc.gpsimd.dma_start(out=wq2r[:], in_=wq2[:])
        nc.gpsimd.dma_start(out=wv2r[:], in_=wv2[:])
        nc.gpsimd.dma_start(out=wor[:], in_=wo[:])
        nc.gpsimd.dma_start(out=caus[:], in_=causal[:])
        ones4 = perm.tile([128, 4], F32, tag="ones4")
        nc.gpsimd.memset(ones4[:], 1.0)
        ones8 = perm.tile([128, 8], F32, tag="ones8")
        nc.gpsimd.memset(ones8[:], 1.0)

        # --- persistent activations ---
        XT = [perm.tile([128, TN], F32R, tag=f"XT{b}", name=f"XT{b}") for b in range(B)]
        KT = [perm.tile([128, TT], F32R, tag=f"KT{b}", name=f"KT{b}") for b in range(B)]
        QT = [perm.tile([128, TN], F32R, tag=f"QT{b}", name=f"QT{b}") for b in range(B)]
        VT = [[perm.tile([128, KT_N * (HS + 1)], F32R, tag=f"VT{b}{h}", name=f"VT{b}{h}")
               for h in range(HPC)] for b in range(B)]

        def vt_view(b, h):
            return VT[b][h][:].rearrange("p (kt c) -> p kt c", c=HS + 1)

        # ---------------- Phase A: projections / layouts ----------------
        with tc.tile_pool(name="psA", bufs=2, space="PSUM") as psA:
            for b in range(B):
                # X^T for this core's 128 embed dims (2 heads x 64)
                xv = xh[b * TN:(b + 1) * TN, :].rearrange(
                    "(t p) e -> p t e", p=128)
                for tg in range(4):
                    xsg = work.tile([128, 4, 128], F32, tag="xsg")
                    nc.sync.dma_start(out=xsg[:], in_=xv[:, tg * 4:(tg + 1) * 4, :])
                    for i in range(4):
                        t = tg * 4 + i
                        tp = psA.tile([128, 128], F32, tag="tp")
                        nc.tensor.transpose(tp[:], xsg[:, i, :], ident[:])
                        if t % 2 == 0:
                            nc.vector.tensor_copy(
                                XT[b][:, t * 128:(t + 1) * 128], tp[:])
                        else:
                            nc.scalar.copy(
                                XT[b][:, t * 128:(t + 1) * 128], tp[:])
                # past K^T (head pair stacked on partitions)
                for tg in range(4):
                    ksg = work.tile([128, 4, 128], F32, tag="ksg")
                    for h in range(HPC):
                        src = pk[b, h, tg * 512:(tg + 1) * 512, :].rearrange(
                            "(kt p) d -> p kt d", p=128)
                        nc.sync.dma_start(out=ksg[:, :, 64 * h:64 * h + 64],
                                          in_=src)
                    for i in range(4):
                        t = tg * 4 + i
                        tp = psA.tile([128, 128], F32, tag="tp")
                        nc.tensor.transpose(tp[:], ksg[:, i, :], ident[:])
                        if t % 2 == 0:
                            nc.vector.tensor_copy(
                                KT[b][:, t * 128:(t + 1) * 128], tp[:])
                        else:
                            nc.scalar.copy(
                                KT[b][:, t * 128:(t + 1) * 128], tp[:])
                # new K^T and Q^T (col-tiled head pair)
                for kc in range(TN // 512):
                    sl = slice(kc * 512, (kc + 1) * 512)
                    nsl = slice(TP + kc * 512, TP + (kc + 1) * 512)
                    kp0 = psA.tile([64, 512], F32, tag="kqp", name="kp0")
                    kp1 = psA.tile([64, 512], F32, tag="kqp", name="kp1")
                    nc.tensor.matmul(kp0[:], wk2r[0:64, :], XT[b][0:64, sl],
                                     start=True, stop=True, tile_position=(0, 0))
                    nc.tensor.matmul(kp1[:], wk2r[64:128, :],
                                     XT[b][64:128, sl], start=True, stop=True,
                                     tile_position=(64, 0))
                    nc.vector.tensor_copy(KT[b][0:64, nsl], kp0[:])
                    nc.scalar.copy(KT[b][64:128, nsl], kp1[:])
                    qp0 = psA.tile([64, 512], F32, tag="kqp", name="qp0")
                    qp1 = psA.tile([64, 512], F32, tag="kqp", name="qp1")
                    nc.tensor.matmul(qp0[:], wq2r[0:64, :], XT[b][0:64, sl],
                                     start=True, stop=True, tile_position=(0, 0))
                    nc.tensor.matmul(qp1[:], wq2r[64:128, :],
                                     XT[b][64:128, sl], start=True, stop=True,
                                     tile_position=(64, 0))
                    nc.vector.tensor_copy(QT[b][0:64, sl], qp0[:])
                    nc.scalar.copy(QT[b][64:128, sl], qp1[:])
                # past V into V~ tiles (fp32r cast during SWDGE DMA) + ones col
                for h in range(HPC):
                    for g in range(4):
                        vv = vt_view(b, h)[:, g * 4:(g + 1) * 4, :]
                        src = pv[b, h, g * 512:(g + 1) * 512, :].rearrange(
                            "(kt p) d -> p kt d", p=128)
                        nc.gpsimd.dma_start(out=vv[:, :, 0:HS], in_=src)
                        nc.vector.tensor_copy(
                            vv[:, :, HS:HS + 1],
                            ones4[:].rearrange("p (kt c) -> p kt c", c=1))
                # new K / V natural layout (knew/vnew outputs + V~ tiles)
                for g in range(2):
                    knp = [psA.tile([128, 512], F32, tag="knp", name="knp") for _ in range(HPC)]
                    vnp = [psA.tile([128, 512], F32, tag="vnp", name="vnp") for _ in range(HPC)]
                    for i in range(8):
                        kt = g * 8 + i
                        ksl = slice(kt * 128, (kt + 1) * 128)
                        osl = slice(i * 64, (i + 1) * 64)
                        for h in range(HPC):
                            hsl = slice(64 * h, 64 * h + 64)
                            nc.tensor.matmul(knp[h][:, osl], XT[b][hsl, ksl],
                                             wk2r[hsl, :], start=True, stop=True,
                                             tile_position=(64 * h, 0))
                            nc.tensor.matmul(vnp[h][:, osl], XT[b][hsl, ksl],
                                             wv2r[hsl, :], start=True, stop=True,
                                             tile_position=(64 * h, 0))
                    for h in range(HPC):
                        kns = work.tile([128, 512], F32, tag="kns")
                        nc.scalar.copy(kns[:], knp[h][:])
                        dst = knew[b, h].rearrange("(kt p) d -> p kt d", p=128)
                        nc.sync.dma_start(
                            out=dst[:, g * 8:(g + 1) * 8, :],
                            in_=kns[:].rearrange("p (kt d) -> p kt d", d=64))
                        # V: evac rounded into V~ strided cols, then DMA out
                        vv = vt_view(b, h)[:, 16 + g * 8:16 + (g + 1) * 8, :]
                        nc.vector.tensor_copy(
                            vv[:, :, 0:HS],
                            vnp[h][:].rearrange("p (kt d) -> p kt d", d=64))
                        nc.vector.tensor_copy(
                            vv[:, :, HS:HS + 1],
                            ones8[:].rearrange("p (kt c) -> p kt c", c=1))
                        vdst = vnew[b, h].rearrange("(kt p) d -> p kt d", p=128)
                        nc.sync.dma_start(
                            out=vdst[:, g * 8:(g + 1) * 8, :],
                            in_=vv[:, :, 0:HS].bitcast(F32))

        # ---------------- Phase B: attention + partial out-proj ----------
        # Flat (head, ktile) score units packed into 3-bank psum groups,
        # double-buffered so exp (ACT) streams continuously.  Each stream's
        # normalization runs at its end; its 8 out-projection matmuls are
        # deferred and interleaved into the next stream's groups so the PE
        # FIFO never stalls the ACT pipeline.
        with tc.tile_pool(name="psS", bufs=2, space="PSUM") as psS, \
             tc.tile_pool(name="psAV", bufs=2, space="PSUM") as psAV, \
             tc.tile_pool(name="psOP", bufs=2, space="PSUM") as psOP, \
             tc.tile_pool(name="pp", bufs=4) as ppool, \
             tc.tile_pool(name="np", bufs=2) as npool:
            pending = []        # deferred out-proj emitters from prev stream

            def emit_pending(n):
                for _ in range(min(n, len(pending))):
                    pending.pop(0)()

            for b in range(B):
                for qc in range(TN // 512):
                    qsl = slice(qc * 512, (qc + 1) * 512)
                    ktmax = (TP + 512 * (qc + 1)) // 128
                    diag0 = (TP + 512 * qc) // 128
                    # flat (h, kt) units, heads interleaved for PE pairing
                    units = []
                    for kt in range(ktmax):
                        for h in range(HPC):
                            units.append((h, kt))
                    av = [None, None]
                    for g0 in range(0, len(units), GRP):
                        grp = units[g0:g0 + GRP]
                        w = len(grp) * 512
                        sp = psS.tile([128, GRP * 512], F32, tag="sc")
                        for i, (h, kt) in enumerate(grp):
                            hsl = slice(64 * h, 64 * h + 64)
                            nc.tensor.matmul(
                                sp[:, i * 512:(i + 1) * 512],
                                KT[b][hsl, kt * 128:(kt + 1) * 128],
                                QT[b][hsl, qsl], start=True, stop=True,
                                tile_position=(64 * h, 0))
                        pt = ppool.tile([128, GRP * 512], F32R, tag="pt")
                        nc.scalar.activation(pt[:, 0:w], sp[:, 0:w], Exp,
                                             scale=0.125)
                        for i, (h, kt) in enumerate(grp):
                            psl = slice(i * 512, (i + 1) * 512)
                            j = kt - diag0
                            if j >= 0:
                                off = 384 - 128 * j
                                nc.vector.tensor_tensor(
                                    pt[:, psl], pt[:, psl],
                                    caus[:, off:off + 512], MUL)
                            if av[h] is None:
                                av[h] = psAV.tile([128, 512], F32, tag="avop",
                                                  name="av")
                            nc.tensor.matmul(
                                av[h][0:65, :], vt_view(b, h)[:, kt, :],
                                pt[:, psl], start=(kt == 0),
                                stop=(kt == ktmax - 1),
                                skip_group_check=True)
                        emit_pending(1)
                    emit_pending(8)
                    # stream end: denominators + normalize (DVE/GPS work that
                    # runs under the next stream's exps)
                    dp = npool.tile([128, 512], F32, tag="dp")
                    nc.gpsimd.memset(dp[:], 1.0)
                    nc.vector.tensor_copy(dp[0:1, :], av[0][64:65, :])
                    nc.vector.tensor_copy(dp[64:65, :], av[1][64:65, :])
                    oh = npool.tile([128, 512], F32, tag="oh")
                    nc.vector.tensor_copy(oh[0:64, :], av[0][0:64, :])
                    nc.vector.tensor_copy(oh[64:128, :], av[1][0:64, :])
                    rc = npool.tile([128, 512], F32, tag="rc")
                    nc.vector.reciprocal_approx_fast(out=rc[:], in_=dp[:])
                    t64 = npool.tile([1, 512], F32, tag="t64")
                    nc.vector.tensor_copy(t64[:], rc[64:65, :])
                    rb = npool.tile([128, 512], F32, tag="rb")
                    nc.gpsimd.partition_broadcast(rb[:], rc[0:1, :])
                    rb2 = npool.tile([128, 512], F32, tag="rb2")
                    nc.gpsimd.partition_broadcast(rb2[:], t64[0:1, :])
                    ohn = npool.tile([128, 512], F32R, tag="ohn")
                    nc.vector.tensor_tensor(ohn[0:64, :], oh[0:64, :],
                                            rb[0:64, :], MUL)
                    nc.vector.tensor_tensor(ohn[64:128, :], oh[64:128, :],
                                            rb2[64:128, :], MUL)

                    def make_op(b_, qc_, r_, m_, ohn_):
                        def emit():
                            op = psOP.tile([128, 512], F32, tag="op",
                                           name="op")
                            for q2 in range(2):
                                nc.tensor.matmul(
                                    op[:, q2 * 256:(q2 + 1) * 256],
                                    ohn_[:, r_ * 128:(r_ + 1) * 128],
                                    wor[:, m_ * 512 + q2 * 256:
                                        m_ * 512 + (q2 + 1) * 256],
                                    start=True, stop=True)
                            po = npool.tile([128, 512], F32, tag="po")
                            nc.vector.tensor_copy(po[:], op[:])
                            row0 = b_ * TN + qc_ * 512 + r_ * 128
                            nc.sync.dma_start(
                                out=pout[row0:row0 + 128,
                                         m_ * 512:(m_ + 1) * 512],
                                in_=po[:])
                        return emit

                    for r in range(4):
                        for m in range(2):
                            pending.append(make_op(b, qc, r, m, ohn))
            emit_pending(8)
    nc.compile()
    return nc


def _get_program():
    if "nc" not in _cache:
        _cache["nc"] = _build_program()
    return _cache["nc"]


def _numpy_fallback(x, pad_mask, past_k, past_v, Wq, Wk, Wv, Wo, bo):
    xh = x.reshape(B, TN, H, HS)
    q = np.einsum("bthd,ed->bhte", xh, Wq, optimize=True)
    k_new = np.einsum("bthd,ed->bhte", xh, Wk, optimize=True)
    v_new = np.einsum("bthd,ed->bhte", xh, Wv, optimize=True)
    k = np.concatenate([past_k, k_new], axis=2)
    v = np.concatenate([past_v, v_new], axis=2)
    scale = 1.0 / np.sqrt(HS)
    scores = np.einsum("bhqd,bhkd->bhqk", q, k, optimize=True) * scale
    causal = np.arange(TT)[None, :] <= (np.arange(TN)[:, None] + TP)
    mask = causal[None, None] & pad_mask[:, None, None, :]
    scores = np.where(mask, scores, np.float32(-1e30))
    scores -= scores.max(axis=-1, keepdims=True)
    e = np.exp(scores)
    attn = e / e.sum(axis=-1, keepdims=True)
    out = np.einsum("bhqk,bhkd->bhqd", attn, v, optimize=True)
    out = out.transpose(0, 2, 1, 3).reshape(B, TN, D)
    out = out @ Wo.T + bo
    return (out.astype(np.float32), k.astype(np.float32),
            v.astype(np.float32))


def kernel(x, pad_mask, past_k, past_v, Wq, Wk, Wv, Wo, bo):
    x = np.ascontiguousarray(np.asarray(x, dtype=np.float32))
    pad_mask = np.asarray(pad_mask)
    past_k = np.ascontiguousarray(np.asarray(past_k, dtype=np.float32))
    past_v = np.ascontiguousarray(np.asarray(past_v, dtype=np.float32))
    Wq = np.asarray(Wq, dtype=np.float32)
    Wk = np.asarray(Wk, dtype=np.float32)
    Wv = np.asarray(Wv, dtype=np.float32)
    Wo = np.asarray(Wo, dtype=np.float32)
    bo = np.asarray(bo, dtype=np.float32)

    if not bool(pad_mask.all()):
        return _numpy_fallback(x, pad_mask.astype(bool), past_k, past_v,
                               Wq, Wk, Wv, Wo, bo)

    from concourse.bass_utils import run_bass_kernel_spmd

    nc = _get_program()

    # host-side shared prep
    causal = (np.arange(128)[:, None] <= np.arange(896)[None, :] - 384
              ).astype(np.float32)
    wk2 = np.ascontiguousarray(np.vstack([Wk.T, Wk.T]))
    wq2 = np.ascontiguousarray(np.vstack([Wq.T, Wq.T]))
    wv2 = np.ascontiguousarray(np.vstack([Wv.T, Wv.T]))
    xf = x.reshape(B * TN, D)

    in_maps = []
    for c in range(NCORES):
        cs = slice(c * 128, (c + 1) * 128)
        hs = slice(HPC * c, HPC * (c + 1))
        in_maps.append({
            "xh": np.ascontiguousarray(xf[:, cs]),
            "pk": np.ascontiguousarray(past_k[:, hs]),
            "pv": np.ascontiguousarray(past_v[:, hs]),
            "wk2": wk2, "wq2": wq2, "wv2": wv2,
            "wo": np.ascontiguousarray(Wo[:, cs].T),
            "causal": causal,
        })

    _cache["last_in_maps"] = in_maps
    trace = bool(int(os.environ.get("KERNEL_PROFILE", "0")))
    if trace:
        try:
            import ntff_hook
            trace = ntff_hook.install()
        except Exception:
            trace = False
    try:
        res = run_bass_kernel_spmd(nc, in_maps, list(range(NCORES)),
                                   trace=trace)
    except Exception:
        if not trace:
            raise
        res = run_bass_kernel_spmd(nc, in_maps, list(range(NCORES)))
    _cache["last_exec_time_ns"] = res.exec_time_ns
    _cache["last_results_obj"] = res

    out = np.zeros((B * TN, D), dtype=np.float32)
    for c in range(NCORES):
        out += res.results[c]["pout"]
    out += bo[None, :]
    out = out.reshape(B, TN, D)

    k = np.empty((B, H, TT, HS), dtype=np.float32)
    v = np.empty((B, H, TT, HS), dtype=np.float32)
    k[:, :, :TP] = past_k
    v[:, :, :TP] = past_v
    for c in range(NCORES):
        hs = slice(HPC * c, HPC * (c + 1))
        k[:, hs, TP:] = res.results[c]["knew"]
        v[:, hs, TP:] = res.results[c]["vnew"]
    return out, k, v


# revision 16
# speedup vs baseline: 15.8640x; 1.0183x over previous
"""MultiHeadAttentionDecoder kernel for 8 Trainium2 NeuronCores.

Sharding: 8-way tensor parallel over heads (2 heads per core), both batches
and all queries on every core.  Each core:
  - projects Q/K/V for its 2 heads (transposed layouts via PE transpose),
  - runs causal flash-attention (scores kept transposed: [keys, queries]),
    softmax without max-subtraction (scores are O(1) bounded), denominator
    via a ones-column appended to V,
  - computes its partial output projection (row-shard of W_out).
Host sums the 8 partial projections (+bias) and concatenates past K/V with
the new K/V computed on device.  Big matmuls run as float32r (~1e-4 rel).
"""

import os
import sys

import numpy as np

sys.path.insert(0, "/opt/trn_rl_repo")

B, H, TN, TP, HS, D = 2, 16, 2048, 2048, 64, 1024
TT = TN + TP                     # 4096 total keys
NCORES = 8
HPC = H // NCORES                # 2 heads per core
KT_N = TT // 128                 # 32 key tiles of 128
GRP = 2                          # score units per exp group

_cache = {}


def _patch_ldw_opt():
    from concourse import bass_utils as _bu
    if getattr(_bu, "_ldwopt_patched", False):
        return
    _orig = _bu.run_command

    def _run(argv, **kw):
        argv = ["--enable-ldw-opt=true" if a == "--enable-ldw-opt=false"
                else a for a in argv]
        return _orig(argv, **kw)

    _bu.run_command = _run
    _bu._ldwopt_patched = True


def _build_program():
    import concourse.mybir as mybir
    if os.environ.get("KERNEL_LDWOPT", "0") == "1":
        _patch_ldw_opt()
    from concourse import bacc
    from concourse.masks import make_identity
    from concourse.tile import TileContext

    F32 = mybir.dt.float32
    F32R = mybir.dt.float32r
    BF16 = mybir.dt.bfloat16
    Exp = mybir.ActivationFunctionType.Exp
    MUL = mybir.AluOpType.mult

    nc = bacc.Bacc("TRN2", target_bir_lowering=False, debug=False,
                   num_devices=NCORES)

    xh = nc.dram_tensor("xh", [B * TN, 128], F32, kind="ExternalInput")
    pk = nc.dram_tensor("pk", [B, HPC, TP, HS], F32, kind="ExternalInput")
    pv = nc.dram_tensor("pv", [B, HPC, TP, HS], F32, kind="ExternalInput")
    wk2 = nc.dram_tensor("wk2", [128, HS], F32, kind="ExternalInput")
    wq2 = nc.dram_tensor("wq2", [128, HS], F32, kind="ExternalInput")
    wv2 = nc.dram_tensor("wv2", [128, HS], F32, kind="ExternalInput")
    wo = nc.dram_tensor("wo", [128, D], F32, kind="ExternalInput")
    causal = nc.dram_tensor("causal", [128, 896], F32, kind="ExternalInput")
    pout = nc.dram_tensor("pout", [B * TN, D], F32, kind="ExternalOutput")
    knew = nc.dram_tensor("knew", [B, HPC, TN, HS], F32, kind="ExternalOutput")
    vnew = nc.dram_tensor("vnew", [B, HPC, TN, HS], F32, kind="ExternalOutput")

    from contextlib import ExitStack
    with TileContext(nc) as tc, ExitStack() as ctx:
        perm = ctx.enter_context(tc.tile_pool(name="perm", bufs=1))
        work = ctx.enter_context(tc.tile_pool(name="work", bufs=3))
        # --- constants ---
        ident = perm.tile([128, 128], F32, tag="ident")
        make_identity(nc, ident[:])
        wk2r = perm.tile([128, HS], F32R, tag="wk2r")
        wq2r = perm.tile([128, HS], F32R, tag="wq2r")
        wv2r = perm.tile([128, HS], F32R, tag="wv2r")
        wor = perm.tile([128, D], F32R, tag="wor")
        caus = perm.tile([128, 896], BF16, tag="caus")
        nc.gpsimd.dma_start(out=wk2r[:], in_=wk2[:])
        nc.gpsimd.dma_start(out=wq2r[:], in_=wq2[:])
        nc.gpsimd.dma_start(out=wv2r[:], in_=wv2[:])
        nc.gpsimd.dma_start(out=wor[:], in_=wo[:])
        nc.gpsimd.dma_start(out=caus[:], in_=causal[:])
        ones4 = perm.tile([128, 4], F32, tag="ones4")
        nc.gpsimd.memset(ones4[:], 1.0)
        ones8 = perm.tile([128, 8], F32, tag="ones8")
        nc.gpsimd.memset(ones8[:], 1.0)

        # --- persistent activations ---
        XT = [perm.tile([128, TN], F32R, tag=f"XT{b}", name=f"XT{b}") for b in range(B)]
        KT = [perm.tile([128, TT], BF16, tag=f"KT{b}", name=f"KT{b}") for b in range(B)]
        QT = [perm.tile([128, TN], BF16, tag=f"QT{b}", name=f"QT{b}") for b in range(B)]
        VT = [[perm.tile([128, KT_N * (HS + 1)], BF16, tag=f"VT{b}{h}", name=f"VT{b}{h}")
               for h in range(HPC)] for b in range(B)]

        def vt_view(b, h):
            return VT[b][h][:].rearrange("p (kt c) -> p kt c", c=HS + 1)

        # ---------------- Phase A: projections / layouts ----------------
        with tc.tile_pool(name="psA", bufs=2, space="PSUM") as psA:
            for b in range(B):
                # X^T for this core's 128 embed dims (2 heads x 64)
                xv = xh[b * TN:(b + 1) * TN, :].rearrange(
                    "(t p) e -> p t e", p=128)
                for tg in range(4):
                    xsg = work.tile([128, 4, 128], F32, tag="xsg")
                    nc.sync.dma_start(out=xsg[:], in_=xv[:, tg * 4:(tg + 1) * 4, :])
                    for i in range(4):
                        t = tg * 4 + i
                        tp = psA.tile([128, 128], F32, tag="tp")
                        nc.tensor.transpose(tp[:], xsg[:, i, :], ident[:])
                        if t % 2 == 0:
                            nc.vector.tensor_copy(
                                XT[b][:, t * 128:(t + 1) * 128], tp[:])
                        else:
                            nc.scalar.copy(
                                XT[b][:, t * 128:(t + 1) * 128], tp[:])
                # past K^T (head pair stacked on partitions)
                for tg in range(4):
                    ksg = work.tile([128, 4, 128], F32, tag="ksg")
                    for h in range(HPC):
                        src = pk[b, h, tg * 512:(tg + 1) * 512, :].rearrange(
                            "(kt p) d -> p kt d", p=128)
                        nc.sync.dma_start(out=ksg[:, :, 64 * h:64 * h + 64],
                                          in_=src)
                    for i in range(4):
                        t = tg * 4 + i
                        tp = psA.tile([128, 128], F32, tag="tp")
                        nc.tensor.transpose(tp[:], ksg[:, i, :], ident[:])
                        if t % 2 == 0:
                            nc.vector.tensor_copy(
                                KT[b][:, t * 128:(t + 1) * 128], tp[:])
                        else:
                            nc.scalar.copy(
                                KT[b][:, t * 128:(t + 1) * 128], tp[:])
                # new K^T and Q^T (col-tiled head pair)
                for kc in range(TN // 512):
                    sl = slice(kc * 512, (kc + 1) * 512)
                    nsl = slice(TP + kc * 512, TP + (kc + 1) * 512)
                    kp0 = psA.tile([64, 512], F32, tag="kqp", name="kp0")
                    kp1 = psA.tile([64, 512], F32, tag="kqp", name="kp1")
                    nc.tensor.matmul(kp0[:], wk2r[0:64, :], XT[b][0:64, sl],
                                     start=True, stop=True, tile_position=(0, 0))
                    nc.tensor.matmul(kp1[:], wk2r[64:128, :],
                                     XT[b][64:128, sl], start=True, stop=True,
                                     tile_position=(64, 0))
                    nc.vector.tensor_copy(KT[b][0:64, nsl], kp0[:])
                    nc.scalar.copy(KT[b][64:128, nsl], kp1[:])
                    qp0 = psA.tile([64, 512], F32, tag="kqp", name="qp0")
                    qp1 = psA.tile([64, 512], F32, tag="kqp", name="qp1")
                    nc.tensor.matmul(qp0[:], wq2r[0:64, :], XT[b][0:64, sl],
                                     start=True, stop=True, tile_position=(0, 0))
                    nc.tensor.matmul(qp1[:], wq2r[64:128, :],
                                     XT[b][64:128, sl], start=True, stop=True,
                                     tile_position=(64, 0))
                    nc.vector.tensor_copy(QT[b][0:64, sl], qp0[:])
                    nc.scalar.copy(QT[b][64:128, sl], qp1[:])
                # past V into V~ tiles (fp32r cast during SWDGE DMA) + ones col
                for h in range(HPC):
                    for g in range(4):
                        vv = vt_view(b, h)[:, g * 4:(g + 1) * 4, :]
                        src = pv[b, h, g * 512:(g + 1) * 512, :].rearrange(
                            "(kt p) d -> p kt d", p=128)
                        nc.gpsimd.dma_start(out=vv[:, :, 0:HS], in_=src)
                        nc.vector.tensor_copy(
                            vv[:, :, HS:HS + 1],
                            ones4[:].rearrange("p (kt c) -> p kt c", c=1))
                # new K / V natural layout (knew/vnew outputs + V~ tiles)
                for g in range(2):
                    knp = [psA.tile([128, 512], F32, tag="knp", name="knp") for _ in range(HPC)]
                    vnp = [psA.tile([128, 512], F32, tag="vnp", name="vnp") for _ in range(HPC)]
                    for i in range(8):
                        kt = g * 8 + i
                        ksl = slice(kt * 128, (kt + 1) * 128)
                        osl = slice(i * 64, (i + 1) * 64)
                        for h in range(HPC):
                            hsl = slice(64 * h, 64 * h + 64)
                            nc.tensor.matmul(knp[h][:, osl], XT[b][hsl, ksl],
                                             wk2r[hsl, :], start=True, stop=True,
                                             tile_position=(64 * h, 0))
                            nc.tensor.matmul(vnp[h][:, osl], XT[b][hsl, ksl],
                                             wv2r[hsl, :], start=True, stop=True,
                                             tile_position=(64 * h, 0))
                    for h in range(HPC):
                        kns = work.tile([128, 512], F32, tag="kns")
                        nc.scalar.copy(kns[:], knp[h][:])
                        dst = knew[b, h].rearrange("(kt p) d -> p kt d", p=128)
                        nc.sync.dma_start(
                            out=dst[:, g * 8:(g + 1) * 8, :],
                            in_=kns[:].rearrange("p (kt d) -> p kt d", d=64))
                        # V: bf16 into V~ strided cols + separate fp32 out
                        vv = vt_view(b, h)[:, 16 + g * 8:16 + (g + 1) * 8, :]
                        nc.vector.tensor_copy(
                            vv[:, :, 0:HS],
                            vnp[h][:].rearrange("p (kt d) -> p kt d", d=64))
                        nc.vector.tensor_copy(
                            vv[:, :, HS:HS + 1],
                            ones8[:].rearrange("p (kt c) -> p kt c", c=1))
                        vns = work.tile([128, 512], F32, tag="vns")
                        nc.scalar.copy(vns[:], vnp[h][:])
                        vdst = vnew[b, h].rearrange("(kt p) d -> p kt d", p=128)
                        nc.sync.dma_start(
                            out=vdst[:, g * 8:(g + 1) * 8, :],
                            in_=vns[:].rearrange("p (kt d) -> p kt d", d=64))

        # ---------------- Phase B: attention + partial out-proj ----------
        # Flat (head, ktile) score units packed into 3-bank psum groups,
        # double-buffered so exp (ACT) streams continuously.  Each stream's
        # normalization runs at its end; its 8 out-projection matmuls are
        # deferred and interleaved into the next stream's groups so the PE
        # FIFO never stalls the ACT pipeline.
        with tc.tile_pool(name="psS", bufs=2, space="PSUM") as psS, \
             tc.tile_pool(name="psAV", bufs=2, space="PSUM") as psAV, \
             tc.tile_pool(name="psOP", bufs=2, space="PSUM") as psOP, \
             tc.tile_pool(name="pp", bufs=4) as ppool, \
             tc.tile_pool(name="np", bufs=2) as npool:
            pending = []        # deferred out-proj emitters from prev stream

            def emit_pending(n):
                for _ in range(min(n, len(pending))):
                    pending.pop(0)()

            for b in range(B):
                for qc in range(TN // 512):
                    qsl = slice(qc * 512, (qc + 1) * 512)
                    ktmax = (TP + 512 * (qc + 1)) // 128
                    diag0 = (TP + 512 * qc) // 128
                    # flat (h, kt) units, heads interleaved for PE pairing
                    units = []
                    for kt in range(ktmax):
                        for h in range(HPC):
                            units.append((h, kt))
                    av = [None, None]
                    for g0 in range(0, len(units), GRP):
                        grp = units[g0:g0 + GRP]
                        w = len(grp) * 512
                        sp = psS.tile([128, GRP * 512], F32, tag="sc")
                        for i, (h, kt) in enumerate(grp):
                            hsl = slice(64 * h, 64 * h + 64)
                            nc.tensor.matmul(
                                sp[:, i * 512:(i + 1) * 512],
                                KT[b][hsl, kt * 128:(kt + 1) * 128],
                                QT[b][hsl, qsl], start=True, stop=True,
                                tile_position=(64 * h, 0))
                        pt = ppool.tile([128, GRP * 512], BF16, tag="pt")
                        nc.scalar.activation(pt[:, 0:w], sp[:, 0:w], Exp,
                                             scale=0.125)
                        for i, (h, kt) in enumerate(grp):
                            psl = slice(i * 512, (i + 1) * 512)
                            j = kt - diag0
                            if j >= 0:
                                off = 384 - 128 * j
                                nc.vector.tensor_tensor(
                                    pt[:, psl], pt[:, psl],
                                    caus[:, off:off + 512], MUL)
                            if av[h] is None:
                                av[h] = psAV.tile([128, 512], F32, tag="avop",
                                                  name="av")
                            nc.tensor.matmul(
                                av[h][0:65, :], vt_view(b, h)[:, kt, :],
                                pt[:, psl], start=(kt == 0),
                                stop=(kt == ktmax - 1),
                                skip_group_check=True)
                        emit_pending(1)
                    emit_pending(8)
                    # stream end: denominators + normalize (DVE/GPS work that
                    # runs under the next stream's exps)
                    dp = npool.tile([128, 512], F32, tag="dp")
                    nc.gpsimd.memset(dp[:], 1.0)
                    nc.vector.tensor_copy(dp[0:1, :], av[0][64:65, :])
                    nc.vector.tensor_copy(dp[64:65, :], av[1][64:65, :])
                    oh = npool.tile([128, 512], F32, tag="oh")
                    nc.vector.tensor_copy(oh[0:64, :], av[0][0:64, :])
                    nc.vector.tensor_copy(oh[64:128, :], av[1][0:64, :])
                    rc = npool.tile([128, 512], F32, tag="rc")
                    nc.vector.reciprocal_approx_fast(out=rc[:], in_=dp[:])
                    t64 = npool.tile([1, 512], F32, tag="t64")
                    nc.vector.tensor_copy(t64[:], rc[64:65, :])
                    rb = npool.tile([128, 512], F32, tag="rb")
                    nc.gpsimd.partition_broadcast(rb[:], rc[0:1, :])
                    rb2 = npool.tile([128, 512], F32, tag="rb2")
                    nc.gpsimd.partition_broadcast(rb2[:], t64[0:1, :])
                    ohn = npool.tile([128, 512], F32R, tag="ohn")
                    nc.vector.tensor_tensor(ohn[0:64, :], oh[0:64, :],
                                            rb[0:64, :], MUL)
                    nc.vector.tensor_tensor(ohn[64:128, :], oh[64:128, :],
                                            rb2[64:128, :], MUL)

                    def make_op(b_, qc_, r_, m_, ohn_):
                        def emit():
                            op = psOP.tile([128, 512], F32, tag="op",
                                           name="op")
                            for q2 in range(2):
                                nc.tensor.matmul(
                                    op[:, q2 * 256:(q2 + 1) * 256],
                                    ohn_[:, r_ * 128:(r_ + 1) * 128],
                                    wor[:, m_ * 512 + q2 * 256:
                                        m_ * 512 + (q2 + 1) * 256],
                                    start=True, stop=True)
                            po = npool.tile([128, 512], F32, tag="po")
                            nc.vector.tensor_copy(po[:], op[:])
                            row0 = b_ * TN + qc_ * 512 + r_ * 128
                            nc.sync.dma_start(
                                out=pout[row0:row0 + 128,
                                         m_ * 512:(m_ + 1) * 512],
                                in_=po[:])
                        return emit

                    for r in range(4):
                        for m in range(2):
                            pending.append(make_op(b, qc, r, m, ohn))
            emit_pending(8)
    nc.compile()
    return nc


def _get_program():
    if "nc" not in _cache:
        _cache["nc"] = _build_program()
    return _cache["nc"]


def _numpy_fallback(x, pad_mask, past_k, past_v, Wq, Wk, Wv, Wo, bo):
    xh = x.reshape(B, TN, H, HS)
    q = np.einsum("bthd,ed->bhte", xh, Wq, optimize=True)
    k_new = np.einsum("bthd,ed->bhte", xh, Wk, optimize=True)
    v_new = np.einsum("bthd,ed->bhte", xh, Wv, optimize=True)
    k = np.concatenate([past_k, k_new], axis=2)
    v = np.concatenate([past_v, v_new], axis=2)
    scale = 1.0 / np.sqrt(HS)
    scores = np.einsum("bhqd,bhkd->bhqk", q, k, optimize=True) * scale
    causal = np.arange(TT)[None, :] <= (np.arange(TN)[:, None] + TP)
    mask = causal[None, None] & pad_mask[:, None, None, :]
    scores = np.where(mask, scores, np.float32(-1e30))
    scores -= scores.max(axis=-1, keepdims=True)
    e = np.exp(scores)
    attn = e / e.sum(axis=-1, keepdims=True)
    out = np.einsum("bhqk,bhkd->bhqd", attn, v, optimize=True)
    out = out.transpose(0, 2, 1, 3).reshape(B, TN, D)
    out = out @ Wo.T + bo
    return (out.astype(np.float32), k.astype(np.float32),
            v.astype(np.float32))


def kernel(x, pad_mask, past_k, past_v, Wq, Wk, Wv, Wo, bo):
    x = np.ascontiguousarray(np.asarray(x, dtype=np.float32))
    pad_mask = np.asarray(pad_mask)
    past_k = np.ascontiguousarray(np.asarray(past_k, dtype=np.float32))
    past_v = np.ascontiguousarray(np.asarray(past_v, dtype=np.float32))
    Wq = np.asarray(Wq, dtype=np.float32)
    Wk = np.asarray(Wk, dtype=np.float32)
    Wv = np.asarray(Wv, dtype=np.float32)
    Wo = np.asarray(Wo, dtype=np.float32)
    bo = np.asarray(bo, dtype=np.float32)

    if not bool(pad_mask.all()):
        return _numpy_fallback(x, pad_mask.astype(bool), past_k, past_v,
                               Wq, Wk, Wv, Wo, bo)

    from concourse.bass_utils import run_bass_kernel_spmd

    nc = _get_program()

    # host-side shared prep
    causal = (np.arange(128)[:, None] <= np.arange(896)[None, :] - 384
              ).astype(np.float32)
    wk2 = np.ascontiguousarray(np.vstack([Wk.T, Wk.T]))
    wq2 = np.ascontiguousarray(np.vstack([Wq.T, Wq.T]))
    wv2 = np.ascontiguousarray(np.vstack([Wv.T, Wv.T]))
    xf = x.reshape(B * TN, D)

    in_maps = []
    for c in range(NCORES):
        cs = slice(c * 128, (c + 1) * 128)
        hs = slice(HPC * c, HPC * (c + 1))
        in_maps.append({
            "xh": np.ascontiguousarray(xf[:, cs]),
            "pk": np.ascontiguousarray(past_k[:, hs]),
            "pv": np.ascontiguousarray(past_v[:, hs]),
            "wk2": wk2, "wq2": wq2, "wv2": wv2,
            "wo": np.ascontiguousarray(Wo[:, cs].T),
            "causal": causal,
        })

    _cache["last_in_maps"] = in_maps
    trace = bool(int(os.environ.get("KERNEL_PROFILE", "0")))
    if trace:
        try:
            import ntff_hook
            trace = ntff_hook.install()
        except Exception:
            trace = False
    try:
        res = run_bass_kernel_spmd(nc, in_maps, list(range(NCORES)),
                                   trace=trace)
    except Exception:
        if not trace:
            raise
        res = run_bass_kernel_spmd(nc, in_maps, list(range(NCORES)))
    _cache["last_exec_time_ns"] = res.exec_time_ns
    _cache["last_results_obj"] = res

    out = np.zeros((B * TN, D), dtype=np.float32)
    for c in range(NCORES):
        out += res.results[c]["pout"]
    out += bo[None, :]
    out = out.reshape(B, TN, D)

    k = np.empty((B, H, TT, HS), dtype=np.float32)
    v = np.empty((B, H, TT, HS), dtype=np.float32)
    k[:, :, :TP] = past_k
    v[:, :, :TP] = past_v
    for c in range(NCORES):
        hs = slice(HPC * c, HPC * (c + 1))
        k[:, hs, TP:] = res.results[c]["knew"]
        v[:, hs, TP:] = res.results[c]["vnew"]
    return out, k, v
